# revision 39
# baseline (speedup 1.0000x reference)
"""Two-layer GCN + linear head on 8 Trainium2 NeuronCores (Bass/Tile).

Math (per GCN layer, PyG GCNConv with self loops, symmetric norm):
    deg[c]  = 1 + |{e : col_e == c}|          (self loop counted)
    dinv    = 1/sqrt(deg)
    u       = dinv * (x @ W)                  (row-wise pre-scale)
    out[c]  = sum_{e->c} dinv[c] * u[row_e] + dinv[c]^2 * (x @ W)[c] + b
    x1      = relu(out)

Device mapping:
  - Nodes padded to a multiple of 8*128; dst blocks of 128 nodes sharded
    contiguously across 8 cores (49 blocks/core for N=50000).
  - Dense phase sharded: each core computes u for its own blocks in bf16;
    TWO half AllGathers per layer (fired as soon as each half of the dense
    outputs is ready) build the full bf16 table, split in two DRAM halves
    that each fit the int16 dma_gather index range.
  - Scatter phase: edges sorted by (dst block, src half); per 128-edge tile,
    dma_gather (round-robin over 4 SWDGE queues, issued AHEAD chunks early)
    pulls bf16 u rows; a PURE 0/1 one-hot O[e,d] = (col_local_e==d) in fp8
    (exact) is built by DVE once in layer 1 and kept SBUF-resident for
    layer 2 (no DRAM cache). The dst-side dinv factor is deferred: folded
    into the per-partition activation scales of the next dense stage
    (dinv^2 for layer-2's table) and the head's final copy; biases enter
    via rank-1 matmuls with a sqrt(deg) row so the deferred scale cancels.
  - Self loops: per block one matmul against a constant fp8 identity
    (u already carries the src-side dinv factor).
  - Transposed accumulation [f,d] feeds layer-2 dense and the head directly
    as matmul stationary operands (no transposes anywhere).

Host does only index-side prep (shard/sort/pad edge lists, integer degree
counts) — all float math runs on device.
"""
import os
import sys

sys.path.insert(0, "/opt/trn_rl_repo")

import numpy as np

import ml_dtypes

P = 128
NCORES = 8
CH = 8       # tiles per dma_gather call (8*128 = 1024 idxs, SWDGE limit)
AHEAD = 8    # gather chunks issued ahead of consumption (per stream)
GBUFS = AHEAD + 8
NQ = 4       # SWDGE queues, round-robin over gather calls
RUNWAY = 16  # blocks opened ahead (self-loop + stream-0 tiles) per layer
XL = 8       # x^T blocks loaded per DMA in the dense phase


def _ceil_div(a, b):
    return (a + b - 1) // b


def _prep(x, edge_index):
    """Host-side index prep. Returns per-core input dicts + metadata."""
    N, D = x.shape
    assert D == P
    NB = _ceil_div(N, P)
    NB = _ceil_div(NB, NCORES) * NCORES  # blocks multiple of 8
    Npad = NB * P
    NBC = NB // NCORES
    # half A as large as the int16 gather-index range allows (32 blocks/core
    # -> 8*4096 = 32768 rows): half B (gated on the LAST AllGather of each
    # layer) stays small, minimizing the end-of-layer bubble.
    NBH = [min(NBC, 32768 // (NCORES * P)), 0]
    NBH[1] = NBC - NBH[0]
    assert NBH[1] * NCORES * P <= 32768
    HSZ = [NBH[0] * P, NBH[1] * P]  # per-core rows in each half table
    assert NCORES * HSZ[0] <= 32768 and NCORES * HSZ[1] <= 32768

    row = np.asarray(edge_index[0], dtype=np.int64)
    col = np.asarray(edge_index[1], dtype=np.int64)

    deg = np.bincount(col, minlength=Npad).astype(np.int64) + 1  # + self loop
    deg[N:] = 1

    blk = (col >> 7).astype(np.int64)
    core = blk // NBC
    loc = blk % NBC
    csrc = row // (NBC * P)
    j = row - csrc * (NBC * P)

    # per (core, local block) totals -> per-core block permutation sorted by
    # edge count so the cross-core slot max is tight
    keyb = core * NBC + loc
    tot = np.bincount(keyb, minlength=NCORES * NBC).reshape(NCORES, NBC)
    perm = np.argsort(-tot, axis=1, kind="stable")  # [core, slot] -> local blk
    inv = np.empty_like(perm)
    np.put_along_axis(inv, perm, np.arange(NBC)[None, :], axis=1)

    # source table position: per-core slot order, split in two halves
    spos = inv[csrc, j // P]
    stream = (spos >= NBH[0]).astype(np.int64)
    gidx = np.where(
        stream == 0,
        csrc * HSZ[0] + spos * P + (j % P),
        csrc * HSZ[1] + (spos - NBH[0]) * P + (j % P),
    )
    sloc = inv[core, loc]  # dst slot index

    # order edges by (dst core, dst slot, src half, src)
    order = np.lexsort((gidx, stream, sloc, core))
    gidx_s, col_s = gidx[order], col[order]
    core_s, loc_s, str_s = core[order], sloc[order], stream[order]

    # per (core, dst slot, stream) counts -> shared slot table
    key = (core_s * NBC + loc_s) * 2 + str_s
    cnt = np.bincount(key, minlength=NCORES * NBC * 2).reshape(NCORES, NBC, 2)
    slots = _ceil_div(cnt, P).max(axis=0)  # [NBC(slots), 2]
    ntiles = [int(slots[:, s].sum()) for s in (0, 1)]
    tstart = np.zeros((NBC + 1, 2), dtype=np.int64)
    tstart[1:] = np.cumsum(slots, axis=0)

    starts = np.zeros(NCORES * NBC * 2 + 1, dtype=np.int64)
    starts[1:] = np.cumsum(cnt.reshape(-1))

    cores = []
    for c in range(NCORES):
        idx = [np.zeros(max(ntiles[s], 1) * P, dtype=np.int64) for s in (0, 1)]
        colv = [np.full(max(ntiles[s], 1) * P, 999, dtype=np.int64) for s in (0, 1)]
        for i in range(NBC):
            for s in (0, 1):
                k = (c * NBC + i) * 2 + s
                lo, hi = starts[k], starts[k + 1]
                o0 = tstart[i, s] * P
                idx[s][o0 : o0 + hi - lo] = gidx_s[lo:hi]
                colv[s][o0 : o0 + hi - lo] = col_s[lo:hi] & 127
        colstream = np.concatenate([colv[0][: ntiles[0] * P], colv[1][: ntiles[1] * P]])

        def wrap16(v):  # [ntiles*128] -> [16, n/16] replicated to 128 rows
            w = v.astype(np.int16).reshape(-1, 16).T
            return np.tile(w, (8, 1)).copy()

        def lanes(v, dt):  # [ntiles*128] -> [128, ntiles] (lane-major columns)
            return np.ascontiguousarray(v.reshape(-1, P).T.astype(dt))

        own = deg[c * NBC * P : (c + 1) * NBC * P].reshape(NBC, P)
        deg_own = own[perm[c]].reshape(NBC, P).T.astype(np.float32)  # [128, NBC]

        xp = np.zeros((NBC, P, P), dtype=np.float32)
        realn = min(max(N - c * NBC * P, 0), NBC * P)
        xp.reshape(-1, P)[:realn] = x[c * NBC * P : c * NBC * P + realn]
        xpad = xp[perm[c]].reshape(NBC * P, P)

        cores.append(
            dict(
                xT_shard=np.ascontiguousarray(xpad.T).astype(ml_dtypes.bfloat16),
                idxA=wrap16(idx[0]),
                idxB=wrap16(idx[1]),
                colstream=lanes(colstream, np.float32),
                deg_own=np.ascontiguousarray(deg_own),  # [128, NBC]
                rdeg=np.sqrt(deg_own.T).reshape(1, NBC * P).astype(
                    ml_dtypes.bfloat16
                ),  # [1, NBC*P] sqrt(deg) row, slot order
            )
        )
    meta = dict(
        N=N, Npad=Npad, NB=NB, NBC=NBC, NBH=NBH, HSZ=HSZ,
        ntilesA=ntiles[0], ntilesB=ntiles[1],
        slots=slots, tstart=tstart, perm=perm,
    )
    return cores, meta


def _build_program(meta, with_bias_gcn):
    """Emit the SPMD bass program (identical for all cores)."""
    from concourse import bacc, mybir
    from concourse.tile import TileContext
    from contextlib import ExitStack

    f32 = mybir.dt.float32
    bf16 = mybir.dt.bfloat16
    f8 = mybir.dt.float8e4
    i16 = mybir.dt.int16
    i32 = mybir.dt.int32
    AF = mybir.ActivationFunctionType
    OP = mybir.AluOpType

    NBC, NBH, HSZ = meta["NBC"], meta["NBH"], meta["HSZ"]
    nA, nB = meta["ntilesA"], meta["ntilesB"]
    ntiles = [nA, nB]
    slots = meta["slots"]
    tstart = meta["tstart"]
    ncols = nA + nB
    nchunks = [_ceil_div(nA, CH), _ceil_div(nB, CH)]

    nc = bacc.Bacc(
        "TRN2",
        target_bir_lowering=False,
        num_devices=NCORES,
        # SBUF descriptor carveout: 64KB/partition (ring = 4096 descs/queue,
        # ~4 in-flight 1024-idx gathers per queue) — frees 64KB of SBUF for
        # deeper gather/one-hot pipeline pools vs the 128KB default.
        dynamic_dma_scratch_size=65536,
        num_swdge_queues=NQ,
    )

    xT = nc.declare_dram_parameter("xT_shard", [P, NBC * P], bf16, isOutput=False)
    W1d = nc.declare_dram_parameter("W1", [P, P], bf16, isOutput=False)
    W2d = nc.declare_dram_parameter("W2", [P, P], bf16, isOutput=False)
    Wld = nc.declare_dram_parameter("Wl", [2 * P, P], bf16, isOutput=False)
    b1d = nc.declare_dram_parameter("b1", [1, P], bf16, isOutput=False)
    b2d = nc.declare_dram_parameter("b2", [1, P], bf16, isOutput=False)
    bld = nc.declare_dram_parameter("bl", [1, P], bf16, isOutput=False)
    idxAd = nc.declare_dram_parameter("idxA", [P, max(nA, 1) * 8], i16, isOutput=False)
    idxBd = nc.declare_dram_parameter("idxB", [P, max(nB, 1) * 8], i16, isOutput=False)
    cold = nc.declare_dram_parameter("colstream", [P, ncols], f32, isOutput=False)
    degod = nc.declare_dram_parameter("deg_own", [P, NBC], f32, isOutput=False)
    rdegd = nc.declare_dram_parameter("rdeg", [1, NBC * P], bf16, isOutput=False)
    outd = nc.declare_dram_parameter("out_shard", [NBC * P, P], f32, isOutput=True)

    ag_in = [
        [nc.dram_tensor(f"ag{L}_in_h{h}", [HSZ[h], P], bf16) if NBH[h] else None
         for h in (0, 1)]
        for L in (0, 1)
    ]
    ag_out = [
        [
            nc.dram_tensor(
                f"ag{L}_out_h{h}", [NCORES * HSZ[h], P], bf16, addr_space="Shared"
            ) if NBH[h] else None
            for h in (0, 1)
        ]
        for L in (0, 1)
    ]

    def _emit(tc, ctx):
        const = ctx.enter_context(tc.tile_pool(name="const", bufs=1))
        sb = ctx.enter_context(tc.tile_pool(name="sb", bufs=3))
        gbufs = ctx.enter_context(tc.tile_pool(name="gbufs", bufs=GBUFS))
        ohp = ctx.enter_context(tc.tile_pool(name="ohp", bufs=96))
        psum = ctx.enter_context(tc.tile_pool(name="psum", bufs=6, space="PSUM"))
        psd = ctx.enter_context(tc.tile_pool(name="psd", bufs=1, space="PSUM"))

        # --- constants / streams ---
        iota_i = const.tile([P, P], i32)
        nc.gpsimd.iota(iota_i[:], pattern=[[1, P]], base=0, channel_multiplier=0)
        iota_bf = const.tile([P, P], bf16)
        nc.vector.tensor_copy(out=iota_bf[:], in_=iota_i[:])
        lane_i = const.tile([P, 1], i32)
        nc.gpsimd.iota(lane_i[:], pattern=[[1, 1]], base=0, channel_multiplier=1)
        lane_f = const.tile([P, 1], f32)
        nc.vector.tensor_copy(out=lane_f[:], in_=lane_i[:])

        W1 = const.tile([P, P], bf16)
        W2 = const.tile([P, P], bf16)
        Wl = const.tile([P, 2 * P], bf16)
        nc.sync.dma_start(out=W1[:], in_=W1d[:])
        nc.sync.dma_start(out=W2[:], in_=W2d[:])
        nc.sync.dma_start(out=Wl[:, 0:P], in_=Wld[0:P, :])
        nc.sync.dma_start(out=Wl[:, P : 2 * P], in_=Wld[P : 2 * P, :])

        # bias tiles (row 0 = bias vector)
        btile = []
        for bi, bd in enumerate((b1d, b2d, bld)):
            t = const.tile([P, P], bf16, tag=f"bias{bi}", name=f"bias{bi}")
            nc.vector.memset(t[:], 0.0)
            nc.sync.dma_start(out=t[0:1, :], in_=bd[:])
            btile.append(t)

        idxs = [
            const.tile([P, max(n, 1) * 8], i16, tag=f"idx{s}", name=f"idx{s}")
            for s, n in ((0, nA), (1, nB))
        ]
        nc.sync.dma_start(out=idxs[0][:], in_=idxAd[:])
        nc.sync.dma_start(out=idxs[1][:], in_=idxBd[:])

        colst = const.tile([P, ncols], f32)
        nc.sync.dma_start(out=colst[:], in_=cold[:])

        dinvo = const.tile([P, NBC], f32)
        nc.sync.dma_start(out=dinvo[:], in_=degod[:])
        nc.scalar.activation(out=dinvo[:], in_=dinvo[:], func=AF.Sqrt)
        nc.vector.reciprocal(out=dinvo[:], in_=dinvo[:])
        dinvo2 = const.tile([P, NBC], f32)  # dinv^2, layer-2 dense scale
        nc.scalar.activation(out=dinvo2[:], in_=dinvo[:], func=AF.Square)

        # sqrt(deg) row (partition 0), slot order; [1, 128] slices feed
        # rank-1 bias matmuls
        rdeg = const.tile([1, NBC * P], bf16)
        nc.sync.dma_start(out=rdeg[:], in_=rdegd[:])

        # constant 0/1 identity (self-loop rhs)
        ident = const.tile([P, P], bf16)
        nc.vector.tensor_single_scalar(
            out=ident[:], in_=iota_bf[:], scalar=lane_f[:, 0:1], op=OP.is_equal
        )

        # persistent per-core tiles
        u_bf = const.tile([P, NBC * P], bf16)  # u blocks [node, f]
        x1T = const.tile([P, NBC * P], bf16)   # a1 = relu(acc1) [f, node]

        def dense_block(b, src_lhsT, W, layer):
            """u[b] = scale_own[b] * (src @ W) -> u_bf (bf16, [node, f])."""
            ps = psd.tile([P, P], f32, space="PSUM", tag="psd")
            nc.tensor.matmul(ps[:], lhsT=src_lhsT, rhs=W[:], start=True, stop=True)
            nc.scalar.activation(
                out=u_bf[:, b * P : (b + 1) * P], in_=ps[:], func=AF.Copy,
                scale=(dinvo if layer == 0 else dinvo2)[:, b : b + 1],
            )

        def send_half(layer, h):
            """DMA u_bf half -> ag_in, AllGather into the half table."""
            c0 = 0 if h == 0 else NBH[0]
            nb = NBH[h]
            src = u_bf[:, c0 * P : (c0 + nb) * P].rearrange("p (i f) -> p i f", f=P)
            dst = ag_in[layer][h][:].rearrange("(i p) f -> p i f", p=P)
            nc.sync.dma_start(out=dst, in_=src)
            nc.gpsimd.collective_compute(
                "AllGather", mybir.AluOpType.bypass,
                replica_groups=[list(range(NCORES))],
                ins=[ag_in[layer][h][:]], outs=[ag_out[layer][h][:]],
            )

        qctr = [0]

        def scatter_layer(layer, post_fn):
            """Message passing for one layer; post_fn(b, acc) consumes the
            accumulated transposed block. Gathers are issued AHEAD chunks
            early, round-robin over the SWDGE queues. Both layers build the
            0/1 fp8 one-hots on DVE into a small rotating pool (cheap:
            single-op is_equal against the iota constant)."""
            issued = [[], []]  # stream -> list of gbuf tiles

            def ensure(s, cid):
                # hysteresis: only top up when close to starvation, then issue
                # a burst — batched gather dispatches keep all 4 SWDGE queues
                # busy concurrently instead of trickling one call at a time
                if len(issued[s]) > min(cid + 2, nchunks[s] - 1):
                    return
                while len(issued[s]) <= min(cid + AHEAD, nchunks[s] - 1):
                    c0 = len(issued[s])
                    ch = min(CH, ntiles[s] - c0 * CH)
                    g = gbufs.tile([P, CH, P], bf16, tag=f"g{s}")
                    nc.gpsimd.dma_gather(
                        out_ap=g[:, 0:ch, :],
                        in_ap=ag_out[layer][s][:],
                        idxs_ap=idxs[s][:, c0 * CH * 8 : (c0 * CH + ch) * 8],
                        num_idxs=ch * P,
                        num_idxs_reg=ch * P,
                        elem_size=P,
                        queue_num=qctr[0] % NQ,
                    )
                    qctr[0] += 1
                    issued[s].append(g)

            def oh_src(gcol):
                o = ohp.tile([P, P], bf16, tag="oh")
                nc.vector.tensor_single_scalar(
                    out=o[:], in_=iota_bf[:],
                    scalar=colst[:, gcol : gcol + 1], op=OP.is_equal,
                )
                return o[:]

            def emit_tiles(b, s, acc, k, nmm, stop_last):
                for t in range(tstart[b, s], tstart[b + 1, s]):
                    cid = t // CH
                    ensure(s, cid)
                    g = issued[s][cid]
                    o = oh_src(nA * s + t)
                    k += 1
                    nc.tensor.matmul(
                        acc, lhsT=g[:, t % CH, :], rhs=o,
                        start=False, stop=(stop_last and k == nmm),
                    )
                return k

            state = {}
            acc4s = {}

            def open_block(b):
                """Open the PSUM bank + self-loop matmul only. Tile matmuls
                (and their gather issuance) are deferred to close_block so the
                gpsimd gather queue stays in consumption order — emitting
                far-future stream-A gathers here made the in-order queue head
                wait on far-future matmuls, stalling every gather behind it."""
                if b % 4 == 0:
                    acc4s[b // 4] = psum.tile(
                        [P, 4 * P], f32, space="PSUM", tag="acc4", name="acc4"
                    )
                acc = acc4s[b // 4][:, (b % 4) * P : (b % 4 + 1) * P]
                nmm = int(slots[b, 0] + slots[b, 1])
                # one accumulation group per PSUM bank (4 blocks): start only
                # on the bank's first matmul, stop only on its last
                nc.tensor.matmul(
                    acc, lhsT=u_bf[:, b * P : (b + 1) * P], rhs=ident[:],
                    start=(b % 4 == 0), stop=False,
                )
                state[b] = (acc, 0, nmm)

            def close_block(b):
                acc, k, nmm = state.pop(b)
                bank_last = (b % 4 == 3) or (b == NBC - 1)
                k = emit_tiles(b, 0, acc, k, nmm, False)
                if with_bias_gcn or slots[b, 1] == 0:
                    emit_tiles(b, 1, acc, k, nmm, False)
                    # rank-1: acc[f,d] += bias[f] * sqrt(deg)[d] (cancels the
                    # deferred dinv[d]); zero bias rows make this a no-op
                    # stop-carrier when stream B is empty.
                    nc.tensor.matmul(
                        acc, lhsT=btile[layer][0:1, :],
                        rhs=rdeg[0:1, b * P : (b + 1) * P],
                        start=False, stop=bank_last,
                    )
                else:
                    emit_tiles(b, 1, acc, k, nmm, bank_last)
                if bank_last:
                    k4 = b // 4
                    a4 = acc4s.pop(k4)
                    for bb in range(k4 * 4, b + 1):
                        post_fn(bb, a4[:, (bb % 4) * P : (bb % 4 + 1) * P])

            # prime the gather pipeline: issue the first AHEAD chunks of both
            # streams before any consumer matmuls, so all 4 SWDGE queues fill
            ensure(0, 0)
            if nchunks[1]:
                ensure(1, 0)

            run = min(RUNWAY, NBC)
            for b in range(run):
                open_block(b)
            for b in range(NBC):
                close_block(b)
                if b + run < NBC:
                    open_block(b + run)

        phase = os.environ.get("KERNEL_PHASE", "full")

        # ---------- layer 1 dense + half AllGathers ----------
        lx = None
        for b in range(NBC):
            if b % XL == 0:
                nxt = min(XL, NBC - b)
                lx = sb.tile([P, XL * P], bf16, tag="xT_in", name="lx")
                nc.sync.dma_start(
                    out=lx[:, 0 : nxt * P], in_=xT[:, b * P : (b + nxt) * P]
                )
            dense_block(b, lx[:, (b % XL) * P : (b % XL + 1) * P], W1, 0)
            if b == NBH[0] - 1:
                send_half(0, 0)
        if NBH[1]:
            send_half(0, 1)
        if phase == "dense":
            for b in range(NBC):
                z = sb.tile([P, P], f32, tag="out_t")
                nc.vector.tensor_copy(out=z[:], in_=u_bf[:, b * P : (b + 1) * P])
                nc.sync.dma_start(out=outd[b * P : (b + 1) * P, :], in_=z[:])
            return

        # ---------- layer 1 scatter -> x1T = a1 (+ layer 2 dense) ----------
        def post1(b, acc):
            nc.scalar.activation(
                out=x1T[:, b * P : (b + 1) * P], in_=acc, func=AF.Relu
            )
            dense_block(b, x1T[:, b * P : (b + 1) * P], W2, 1)
            if b == NBH[0] - 1:
                send_half(1, 0)
            elif b == NBC - 1 and NBH[1]:
                send_half(1, 1)

        scatter_layer(0, post1)
        if phase == "l1":
            for b in range(NBC):
                z = sb.tile([P, P], f32, tag="out_t")
                nc.vector.tensor_copy(out=z[:], in_=x1T[:, b * P : (b + 1) * P])
                nc.sync.dma_start(out=outd[b * P : (b + 1) * P, :], in_=z[:])
            return

        # ---------- layer 2 scatter -> head ----------
        def post2(b, acc):
            x2T = sb.tile([P, P], bf16, tag="x2T")
            nc.scalar.activation(out=x2T[:], in_=acc, func=AF.Relu)
            ph = psd.tile([P, P], f32, space="PSUM", tag="ph")
            nc.tensor.matmul(
                ph[:], lhsT=x1T[:, b * P : (b + 1) * P], rhs=Wl[:, 0:P],
                start=True, stop=False,
            )
            nc.tensor.matmul(
                ph[:], lhsT=x2T[:], rhs=Wl[:, P : 2 * P], start=False, stop=False
            )
            # rank-1 bias: ph[d,o] += sqrt(deg)[d] * bl[o]; the final copy's
            # dinv[d] scale turns this into + bl and the x1/x2 terms into
            # their properly normalized values.
            nc.tensor.matmul(
                ph[:], lhsT=rdeg[0:1, b * P : (b + 1) * P],
                rhs=btile[2][0:1, :], start=False, stop=True,
            )
            ot = sb.tile([P, P], f32, tag="out_t")
            nc.scalar.activation(
                out=ot[:], in_=ph[:], func=AF.Copy, scale=dinvo[:, b : b + 1]
            )
            nc.sync.dma_start(out=outd[b * P : (b + 1) * P, :], in_=ot[:])

        scatter_layer(1, post2)

    with TileContext(nc) as tc, ExitStack() as ctx:
        _emit(tc, ctx)

    nc.compile()
    return nc


def _unpermute(meta, shards):
    """Scatter per-core slot-ordered out rows back to natural block order."""
    NBC, Npad, N, perm = meta["NBC"], meta["Npad"], meta["N"], meta["perm"]
    out = np.empty((Npad, P), np.float32)
    ob = out.reshape(-1, P, P)
    for c in range(NCORES):
        ob[c * NBC + perm[c]] = shards[c].reshape(NBC, P, P)
    return np.ascontiguousarray(out[:N])


def kernel(x, edge_index, W1, b1, W2, b2, Wl, bl):
    x = np.asarray(x, dtype=np.float32)
    cores, meta = _prep(x, np.asarray(edge_index))
    with_bias_gcn = bool(np.any(b1) or np.any(b2))

    nc = _build_program(meta, with_bias_gcn)

    bf = ml_dtypes.bfloat16
    shared = dict(
        W1=np.asarray(W1, np.float32).astype(bf),
        W2=np.asarray(W2, np.float32).astype(bf),
        Wl=np.asarray(Wl, np.float32).astype(bf),
        b1=np.asarray(b1, np.float32).astype(bf).reshape(1, P),
        b2=np.asarray(b2, np.float32).astype(bf).reshape(1, P),
        bl=np.asarray(bl, np.float32).astype(bf).reshape(1, P),
    )
    in_maps = [{**c, **shared} for c in cores]
    N = meta["N"]

    if os.environ.get("KERNEL_SIM"):
        from concourse.bass_interp import MultiCoreSim

        sim = MultiCoreSim(nc, NCORES)
        for i in range(NCORES):
            for k, v in in_maps[i].items():
                sim.cores[i].tensor(k)[:] = v
        sim.simulate()
        out = _unpermute(
            meta,
            [np.asarray(sim.cores[i].tensor("out_shard")) for i in range(NCORES)],
        )
        return out

    from concourse.bass_utils import run_bass_kernel_spmd

    trace = bool(int(os.environ.get("KERNEL_TRACE", "0")))
    if trace:
        try:
            import ntff_shim  # noqa: F401
        except ImportError:
            pass

    br = run_bass_kernel_spmd(nc, in_maps, list(range(NCORES)), trace=trace)
    kernel.last_result = br

    out = _unpermute(meta, [r["out_shard"] for r in br.results])
    return out



# revision 41
# speedup vs baseline: 1.1420x; 1.1420x over previous
"""Two-layer GCN + linear head on 8 Trainium2 NeuronCores (Bass/Tile).

Math (per GCN layer, PyG GCNConv with self loops, symmetric norm):
    deg[c]  = 1 + |{e : col_e == c}|          (self loop counted)
    dinv    = 1/sqrt(deg)
    u       = dinv * (x @ W)                  (row-wise pre-scale)
    out[c]  = sum_{e->c} dinv[c] * u[row_e] + dinv[c]^2 * (x @ W)[c] + b
    x1      = relu(out)

Device mapping:
  - Nodes padded to a multiple of 8*128; dst blocks of 128 nodes sharded
    contiguously across 8 cores (49 blocks/core for N=50000).
  - Dense phase sharded: each core computes u for its own blocks in bf16;
    TWO half AllGathers per layer (fired as soon as each half of the dense
    outputs is ready) build the full bf16 table, split in two DRAM halves
    that each fit the int16 dma_gather index range.
  - Scatter phase: edges sorted by (dst block, src half); per 128-edge tile,
    dma_gather (round-robin over 4 SWDGE queues, issued AHEAD chunks early)
    pulls bf16 u rows; a PURE 0/1 one-hot O[e,d] = (col_local_e==d) in fp8
    (exact) is built by DVE once in layer 1 and kept SBUF-resident for
    layer 2 (no DRAM cache). The dst-side dinv factor is deferred: folded
    into the per-partition activation scales of the next dense stage
    (dinv^2 for layer-2's table) and the head's final copy; biases enter
    via rank-1 matmuls with a sqrt(deg) row so the deferred scale cancels.
  - Self loops: per block one matmul against a constant fp8 identity
    (u already carries the src-side dinv factor).
  - Transposed accumulation [f,d] feeds layer-2 dense and the head directly
    as matmul stationary operands (no transposes anywhere).

Host does only index-side prep (shard/sort/pad edge lists, integer degree
counts) — all float math runs on device.
"""
import os
import sys

sys.path.insert(0, "/opt/trn_rl_repo")

import numpy as np

import ml_dtypes

P = 128
NCORES = 8
CH = 8       # tiles per dma_gather call (8*128 = 1024 idxs, SWDGE limit)
AHEAD = 8    # gather chunks issued ahead of consumption (per stream)
GBUFS = AHEAD + 8
NQ = 4       # SWDGE queues, round-robin over gather calls
RUNWAY = 16  # blocks opened ahead (self-loop + stream-0 tiles) per layer
XL = 8       # x^T blocks loaded per DMA in the dense phase


def _ceil_div(a, b):
    return (a + b - 1) // b


def _prep(x, edge_index):
    """Host-side index prep. Returns per-core input dicts + metadata."""
    N, D = x.shape
    assert D == P
    NB = _ceil_div(N, P)
    NB = _ceil_div(NB, NCORES) * NCORES  # blocks multiple of 8
    Npad = NB * P
    NBC = NB // NCORES
    NBH = [min(_ceil_div(NBC, 2), 32768 // (NCORES * P)), 0]
    NBH[1] = NBC - NBH[0]
    assert NBH[1] * NCORES * P <= 32768
    HSZ = [NBH[0] * P, NBH[1] * P]  # per-core rows in each half table
    assert NCORES * HSZ[0] <= 32768 and NCORES * HSZ[1] <= 32768

    row = np.asarray(edge_index[0], dtype=np.int64)
    col = np.asarray(edge_index[1], dtype=np.int64)

    deg = np.bincount(col, minlength=Npad).astype(np.int64) + 1  # + self loop
    deg[N:] = 1

    blk = (col >> 7).astype(np.int64)
    core = blk // NBC
    loc = blk % NBC
    csrc = row // (NBC * P)
    j = row - csrc * (NBC * P)

    # per (core, local block) totals -> per-core block permutation sorted by
    # edge count so the cross-core slot max is tight
    keyb = core * NBC + loc
    tot = np.bincount(keyb, minlength=NCORES * NBC).reshape(NCORES, NBC)
    perm = np.argsort(-tot, axis=1, kind="stable")  # [core, slot] -> local blk
    inv = np.empty_like(perm)
    np.put_along_axis(inv, perm, np.arange(NBC)[None, :], axis=1)

    # source table position: per-core slot order, split in two halves
    spos = inv[csrc, j // P]
    stream = (spos >= NBH[0]).astype(np.int64)
    gidx = np.where(
        stream == 0,
        csrc * HSZ[0] + spos * P + (j % P),
        csrc * HSZ[1] + (spos - NBH[0]) * P + (j % P),
    )
    sloc = inv[core, loc]  # dst slot index

    # order edges by (dst core, dst slot, src half, src)
    order = np.lexsort((gidx, stream, sloc, core))
    gidx_s, col_s = gidx[order], col[order]
    core_s, loc_s, str_s = core[order], sloc[order], stream[order]

    # per (core, dst slot, stream) counts -> shared slot table
    key = (core_s * NBC + loc_s) * 2 + str_s
    cnt = np.bincount(key, minlength=NCORES * NBC * 2).reshape(NCORES, NBC, 2)
    slots = _ceil_div(cnt, P).max(axis=0)  # [NBC(slots), 2]
    ntiles = [int(slots[:, s].sum()) for s in (0, 1)]
    tstart = np.zeros((NBC + 1, 2), dtype=np.int64)
    tstart[1:] = np.cumsum(slots, axis=0)

    starts = np.zeros(NCORES * NBC * 2 + 1, dtype=np.int64)
    starts[1:] = np.cumsum(cnt.reshape(-1))

    cores = []
    for c in range(NCORES):
        idx = [np.zeros(max(ntiles[s], 1) * P, dtype=np.int64) for s in (0, 1)]
        colv = [np.full(max(ntiles[s], 1) * P, 999, dtype=np.int64) for s in (0, 1)]
        for i in range(NBC):
            for s in (0, 1):
                k = (c * NBC + i) * 2 + s
                lo, hi = starts[k], starts[k + 1]
                o0 = tstart[i, s] * P
                idx[s][o0 : o0 + hi - lo] = gidx_s[lo:hi]
                colv[s][o0 : o0 + hi - lo] = col_s[lo:hi] & 127
        colstream = np.concatenate([colv[0][: ntiles[0] * P], colv[1][: ntiles[1] * P]])

        def wrap16(v):  # [ntiles*128] -> [16, n/16] replicated to 128 rows
            w = v.astype(np.int16).reshape(-1, 16).T
            return np.tile(w, (8, 1)).copy()

        def lanes(v, dt):  # [ntiles*128] -> [128, ntiles] (lane-major columns)
            return np.ascontiguousarray(v.reshape(-1, P).T.astype(dt))

        own = deg[c * NBC * P : (c + 1) * NBC * P].reshape(NBC, P)
        deg_own = own[perm[c]].reshape(NBC, P).T.astype(np.float32)  # [128, NBC]

        xp = np.zeros((NBC, P, P), dtype=np.float32)
        realn = min(max(N - c * NBC * P, 0), NBC * P)
        xp.reshape(-1, P)[:realn] = x[c * NBC * P : c * NBC * P + realn]
        xpad = xp[perm[c]].reshape(NBC * P, P)

        cores.append(
            dict(
                xT_shard=np.ascontiguousarray(xpad.T).astype(ml_dtypes.bfloat16),
                idxA=wrap16(idx[0]),
                idxB=wrap16(idx[1]),
                colstream=lanes(colstream, np.float32),
                deg_own=np.ascontiguousarray(deg_own),  # [128, NBC]
                rdeg=np.sqrt(deg_own.T).reshape(1, NBC * P).astype(
                    ml_dtypes.bfloat16
                ),  # [1, NBC*P] sqrt(deg) row, slot order
            )
        )
    meta = dict(
        N=N, Npad=Npad, NB=NB, NBC=NBC, NBH=NBH, HSZ=HSZ,
        ntilesA=ntiles[0], ntilesB=ntiles[1],
        slots=slots, tstart=tstart, perm=perm,
    )
    return cores, meta


def _build_program(meta, with_bias_gcn):
    """Emit the SPMD bass program (identical for all cores)."""
    from concourse import bacc, mybir
    from concourse.tile import TileContext
    from contextlib import ExitStack

    f32 = mybir.dt.float32
    bf16 = mybir.dt.bfloat16
    f8 = mybir.dt.float8e4
    i16 = mybir.dt.int16
    i32 = mybir.dt.int32
    AF = mybir.ActivationFunctionType
    OP = mybir.AluOpType

    NBC, NBH, HSZ = meta["NBC"], meta["NBH"], meta["HSZ"]
    nA, nB = meta["ntilesA"], meta["ntilesB"]
    ntiles = [nA, nB]
    slots = meta["slots"]
    tstart = meta["tstart"]
    ncols = nA + nB
    nchunks = [_ceil_div(nA, CH), _ceil_div(nB, CH)]

    nc = bacc.Bacc(
        "TRN2",
        target_bir_lowering=False,
        num_devices=NCORES,
        # SBUF descriptor carveout: 64KB/partition (ring = 4096 descs/queue,
        # ~4 in-flight 1024-idx gathers per queue) — frees 64KB of SBUF for
        # deeper gather/one-hot pipeline pools vs the 128KB default.
        dynamic_dma_scratch_size=65536,
        num_swdge_queues=NQ,
    )

    xT = nc.declare_dram_parameter("xT_shard", [P, NBC * P], bf16, isOutput=False)
    W1d = nc.declare_dram_parameter("W1", [P, P], bf16, isOutput=False)
    W2d = nc.declare_dram_parameter("W2", [P, P], bf16, isOutput=False)
    Wld = nc.declare_dram_parameter("Wl", [2 * P, P], bf16, isOutput=False)
    b1d = nc.declare_dram_parameter("b1", [1, P], bf16, isOutput=False)
    b2d = nc.declare_dram_parameter("b2", [1, P], bf16, isOutput=False)
    bld = nc.declare_dram_parameter("bl", [1, P], bf16, isOutput=False)
    idxAd = nc.declare_dram_parameter("idxA", [P, max(nA, 1) * 8], i16, isOutput=False)
    idxBd = nc.declare_dram_parameter("idxB", [P, max(nB, 1) * 8], i16, isOutput=False)
    cold = nc.declare_dram_parameter("colstream", [P, ncols], f32, isOutput=False)
    degod = nc.declare_dram_parameter("deg_own", [P, NBC], f32, isOutput=False)
    rdegd = nc.declare_dram_parameter("rdeg", [1, NBC * P], bf16, isOutput=False)
    outd = nc.declare_dram_parameter("out_shard", [NBC * P, P], f32, isOutput=True)

    ag_in = [
        [nc.dram_tensor(f"ag{L}_in_h{h}", [HSZ[h], P], bf16) if NBH[h] else None
         for h in (0, 1)]
        for L in (0, 1)
    ]
    ag_out = [
        [
            nc.dram_tensor(
                f"ag{L}_out_h{h}", [NCORES * HSZ[h], P], bf16, addr_space="Shared"
            ) if NBH[h] else None
            for h in (0, 1)
        ]
        for L in (0, 1)
    ]

    def _emit(tc, ctx):
        const = ctx.enter_context(tc.tile_pool(name="const", bufs=1))
        sb = ctx.enter_context(tc.tile_pool(name="sb", bufs=3))
        gbufs = ctx.enter_context(tc.tile_pool(name="gbufs", bufs=GBUFS))
        ohp = ctx.enter_context(tc.tile_pool(name="ohp", bufs=96))
        psum = ctx.enter_context(tc.tile_pool(name="psum", bufs=6, space="PSUM"))
        psd = ctx.enter_context(tc.tile_pool(name="psd", bufs=1, space="PSUM"))

        # --- constants / streams ---
        iota_i = const.tile([P, P], i32)
        nc.gpsimd.iota(iota_i[:], pattern=[[1, P]], base=0, channel_multiplier=0)
        iota_bf = const.tile([P, P], bf16)
        nc.vector.tensor_copy(out=iota_bf[:], in_=iota_i[:])
        lane_i = const.tile([P, 1], i32)
        nc.gpsimd.iota(lane_i[:], pattern=[[1, 1]], base=0, channel_multiplier=1)
        lane_f = const.tile([P, 1], f32)
        nc.vector.tensor_copy(out=lane_f[:], in_=lane_i[:])

        W1 = const.tile([P, P], bf16)
        W2 = const.tile([P, P], bf16)
        Wl = const.tile([P, 2 * P], bf16)
        nc.sync.dma_start(out=W1[:], in_=W1d[:])
        nc.sync.dma_start(out=W2[:], in_=W2d[:])
        nc.sync.dma_start(out=Wl[:, 0:P], in_=Wld[0:P, :])
        nc.sync.dma_start(out=Wl[:, P : 2 * P], in_=Wld[P : 2 * P, :])

        # bias tiles (row 0 = bias vector)
        btile = []
        for bi, bd in enumerate((b1d, b2d, bld)):
            t = const.tile([P, P], bf16, tag=f"bias{bi}", name=f"bias{bi}")
            nc.vector.memset(t[:], 0.0)
            nc.sync.dma_start(out=t[0:1, :], in_=bd[:])
            btile.append(t)

        idxs = [
            const.tile([P, max(n, 1) * 8], i16, tag=f"idx{s}", name=f"idx{s}")
            for s, n in ((0, nA), (1, nB))
        ]
        nc.sync.dma_start(out=idxs[0][:], in_=idxAd[:])
        nc.sync.dma_start(out=idxs[1][:], in_=idxBd[:])

        colst = const.tile([P, ncols], f32)
        nc.sync.dma_start(out=colst[:], in_=cold[:])

        dinvo = const.tile([P, NBC], f32)
        nc.sync.dma_start(out=dinvo[:], in_=degod[:])
        nc.scalar.activation(out=dinvo[:], in_=dinvo[:], func=AF.Sqrt)
        nc.vector.reciprocal(out=dinvo[:], in_=dinvo[:])
        dinvo2 = const.tile([P, NBC], f32)  # dinv^2, layer-2 dense scale
        nc.scalar.activation(out=dinvo2[:], in_=dinvo[:], func=AF.Square)

        # sqrt(deg) row (partition 0), slot order; [1, 128] slices feed
        # rank-1 bias matmuls
        rdeg = const.tile([1, NBC * P], bf16)
        nc.sync.dma_start(out=rdeg[:], in_=rdegd[:])

        # constant 0/1 identity (self-loop rhs)
        ident = const.tile([P, P], bf16)
        nc.vector.tensor_single_scalar(
            out=ident[:], in_=iota_bf[:], scalar=lane_f[:, 0:1], op=OP.is_equal
        )

        # persistent per-core tiles
        u_bf = const.tile([P, NBC * P], bf16)  # u blocks [node, f]
        x1T = const.tile([P, NBC * P], bf16)   # a1 = relu(acc1) [f, node]

        def dense_block(b, src_lhsT, W, layer):
            """u[b] = scale_own[b] * (src @ W) -> u_bf (bf16, [node, f])."""
            ps = psd.tile([P, P], f32, space="PSUM", tag="psd")
            nc.tensor.matmul(ps[:], lhsT=src_lhsT, rhs=W[:], start=True, stop=True)
            nc.scalar.activation(
                out=u_bf[:, b * P : (b + 1) * P], in_=ps[:], func=AF.Copy,
                scale=(dinvo if layer == 0 else dinvo2)[:, b : b + 1],
            )

        def send_half(layer, h):
            """DMA u_bf half -> ag_in, AllGather into the half table."""
            c0 = 0 if h == 0 else NBH[0]
            nb = NBH[h]
            src = u_bf[:, c0 * P : (c0 + nb) * P].rearrange("p (i f) -> p i f", f=P)
            dst = ag_in[layer][h][:].rearrange("(i p) f -> p i f", p=P)
            nc.sync.dma_start(out=dst, in_=src)
            nc.gpsimd.collective_compute(
                "AllGather", mybir.AluOpType.bypass,
                replica_groups=[list(range(NCORES))],
                ins=[ag_in[layer][h][:]], outs=[ag_out[layer][h][:]],
            )

        qctr = [0]

        def scatter_layer(layer, post_fn):
            """Message passing for one layer; post_fn(b, acc) consumes the
            accumulated transposed block. Gathers are issued AHEAD chunks
            early, round-robin over the SWDGE queues. Both layers build the
            0/1 fp8 one-hots on DVE into a small rotating pool (cheap:
            single-op is_equal against the iota constant)."""
            issued = [[], []]  # stream -> list of gbuf tiles

            def ensure(s, cid):
                # hysteresis: only top up when close to starvation, then issue
                # a burst — batched gather dispatches keep all 4 SWDGE queues
                # busy concurrently instead of trickling one call at a time
                if len(issued[s]) > min(cid + 2, nchunks[s] - 1):
                    return
                while len(issued[s]) <= min(cid + AHEAD, nchunks[s] - 1):
                    c0 = len(issued[s])
                    ch = min(CH, ntiles[s] - c0 * CH)
                    g = gbufs.tile([P, CH, P], bf16, tag=f"g{s}")
                    nc.gpsimd.dma_gather(
                        out_ap=g[:, 0:ch, :],
                        in_ap=ag_out[layer][s][:],
                        idxs_ap=idxs[s][:, c0 * CH * 8 : (c0 * CH + ch) * 8],
                        num_idxs=ch * P,
                        num_idxs_reg=ch * P,
                        elem_size=P,
                        queue_num=qctr[0] % NQ,
                        single_packet=False,
                    )
                    qctr[0] += 1
                    issued[s].append(g)

            def oh_src(gcol):
                o = ohp.tile([P, P], bf16, tag="oh")
                nc.vector.tensor_single_scalar(
                    out=o[:], in_=iota_bf[:],
                    scalar=colst[:, gcol : gcol + 1], op=OP.is_equal,
                )
                return o[:]

            def emit_tiles(b, s, acc, k, nmm, stop_last):
                for t in range(tstart[b, s], tstart[b + 1, s]):
                    cid = t // CH
                    ensure(s, cid)
                    g = issued[s][cid]
                    o = oh_src(nA * s + t)
                    k += 1
                    nc.tensor.matmul(
                        acc, lhsT=g[:, t % CH, :], rhs=o,
                        start=False, stop=(stop_last and k == nmm),
                    )
                return k

            state = {}
            acc4s = {}

            def open_block(b):
                """Open the PSUM bank + self-loop matmul only. Tile matmuls
                (and their gather issuance) are deferred to close_block so the
                gpsimd gather queue stays in consumption order — emitting
                far-future stream-A gathers here made the in-order queue head
                wait on far-future matmuls, stalling every gather behind it."""
                if b % 4 == 0:
                    acc4s[b // 4] = psum.tile(
                        [P, 4 * P], f32, space="PSUM", tag="acc4", name="acc4"
                    )
                acc = acc4s[b // 4][:, (b % 4) * P : (b % 4 + 1) * P]
                nmm = int(slots[b, 0] + slots[b, 1])
                # one accumulation group per PSUM bank (4 blocks): start only
                # on the bank's first matmul, stop only on its last
                nc.tensor.matmul(
                    acc, lhsT=u_bf[:, b * P : (b + 1) * P], rhs=ident[:],
                    start=(b % 4 == 0), stop=False,
                )
                state[b] = (acc, 0, nmm)

            def close_block(b):
                acc, k, nmm = state.pop(b)
                bank_last = (b % 4 == 3) or (b == NBC - 1)
                k = emit_tiles(b, 0, acc, k, nmm, False)
                if with_bias_gcn or slots[b, 1] == 0:
                    emit_tiles(b, 1, acc, k, nmm, False)
                    # rank-1: acc[f,d] += bias[f] * sqrt(deg)[d] (cancels the
                    # deferred dinv[d]); zero bias rows make this a no-op
                    # stop-carrier when stream B is empty.
                    nc.tensor.matmul(
                        acc, lhsT=btile[layer][0:1, :],
                        rhs=rdeg[0:1, b * P : (b + 1) * P],
                        start=False, stop=bank_last,
                    )
                else:
                    emit_tiles(b, 1, acc, k, nmm, bank_last)
                if bank_last:
                    k4 = b // 4
                    a4 = acc4s.pop(k4)
                    for bb in range(k4 * 4, b + 1):
                        post_fn(bb, a4[:, (bb % 4) * P : (bb % 4 + 1) * P])

            # prime the gather pipeline: issue the first AHEAD chunks of both
            # streams before any consumer matmuls, so all 4 SWDGE queues fill
            ensure(0, 0)
            if nchunks[1]:
                ensure(1, 0)

            run = min(RUNWAY, NBC)
            for b in range(run):
                open_block(b)
            for b in range(NBC):
                close_block(b)
                if b + run < NBC:
                    open_block(b + run)

        phase = os.environ.get("KERNEL_PHASE", "full")

        # ---------- layer 1 dense + half AllGathers ----------
        lx = None
        for b in range(NBC):
            if b % XL == 0:
                nxt = min(XL, NBC - b)
                lx = sb.tile([P, XL * P], bf16, tag="xT_in", name="lx")
                nc.sync.dma_start(
                    out=lx[:, 0 : nxt * P], in_=xT[:, b * P : (b + nxt) * P]
                )
            dense_block(b, lx[:, (b % XL) * P : (b % XL + 1) * P], W1, 0)
            if b == NBH[0] - 1:
                send_half(0, 0)
        if NBH[1]:
            send_half(0, 1)
        if phase == "dense":
            for b in range(NBC):
                z = sb.tile([P, P], f32, tag="out_t")
                nc.vector.tensor_copy(out=z[:], in_=u_bf[:, b * P : (b + 1) * P])
                nc.sync.dma_start(out=outd[b * P : (b + 1) * P, :], in_=z[:])
            return

        # ---------- layer 1 scatter -> x1T = a1 (+ layer 2 dense) ----------
        def post1(b, acc):
            nc.scalar.activation(
                out=x1T[:, b * P : (b + 1) * P], in_=acc, func=AF.Relu
            )
            dense_block(b, x1T[:, b * P : (b + 1) * P], W2, 1)
            if b == NBH[0] - 1:
                send_half(1, 0)
            elif b == NBC - 1 and NBH[1]:
                send_half(1, 1)

        scatter_layer(0, post1)
        if phase == "l1":
            for b in range(NBC):
                z = sb.tile([P, P], f32, tag="out_t")
                nc.vector.tensor_copy(out=z[:], in_=x1T[:, b * P : (b + 1) * P])
                nc.sync.dma_start(out=outd[b * P : (b + 1) * P, :], in_=z[:])
            return

        # ---------- layer 2 scatter -> head ----------
        def post2(b, acc):
            x2T = sb.tile([P, P], bf16, tag="x2T")
            nc.scalar.activation(out=x2T[:], in_=acc, func=AF.Relu)
            ph = psd.tile([P, P], f32, space="PSUM", tag="ph")
            nc.tensor.matmul(
                ph[:], lhsT=x1T[:, b * P : (b + 1) * P], rhs=Wl[:, 0:P],
                start=True, stop=False,
            )
            nc.tensor.matmul(
                ph[:], lhsT=x2T[:], rhs=Wl[:, P : 2 * P], start=False, stop=False
            )
            # rank-1 bias: ph[d,o] += sqrt(deg)[d] * bl[o]; the final copy's
            # dinv[d] scale turns this into + bl and the x1/x2 terms into
            # their properly normalized values.
            nc.tensor.matmul(
                ph[:], lhsT=rdeg[0:1, b * P : (b + 1) * P],
                rhs=btile[2][0:1, :], start=False, stop=True,
            )
            ot = sb.tile([P, P], f32, tag="out_t")
            nc.scalar.activation(
                out=ot[:], in_=ph[:], func=AF.Copy, scale=dinvo[:, b : b + 1]
            )
            nc.sync.dma_start(out=outd[b * P : (b + 1) * P, :], in_=ot[:])

        scatter_layer(1, post2)

    with TileContext(nc) as tc, ExitStack() as ctx:
        _emit(tc, ctx)

    nc.compile()
    return nc


def _unpermute(meta, shards):
    """Scatter per-core slot-ordered out rows back to natural block order."""
    NBC, Npad, N, perm = meta["NBC"], meta["Npad"], meta["N"], meta["perm"]
    out = np.empty((Npad, P), np.float32)
    ob = out.reshape(-1, P, P)
    for c in range(NCORES):
        ob[c * NBC + perm[c]] = shards[c].reshape(NBC, P, P)
    return np.ascontiguousarray(out[:N])


def kernel(x, edge_index, W1, b1, W2, b2, Wl, bl):
    x = np.asarray(x, dtype=np.float32)
    cores, meta = _prep(x, np.asarray(edge_index))
    with_bias_gcn = bool(np.any(b1) or np.any(b2))

    nc = _build_program(meta, with_bias_gcn)

    bf = ml_dtypes.bfloat16
    shared = dict(
        W1=np.asarray(W1, np.float32).astype(bf),
        W2=np.asarray(W2, np.float32).astype(bf),
        Wl=np.asarray(Wl, np.float32).astype(bf),
        b1=np.asarray(b1, np.float32).astype(bf).reshape(1, P),
        b2=np.asarray(b2, np.float32).astype(bf).reshape(1, P),
        bl=np.asarray(bl, np.float32).astype(bf).reshape(1, P),
    )
    in_maps = [{**c, **shared} for c in cores]
    N = meta["N"]

    if os.environ.get("KERNEL_SIM"):
        from concourse.bass_interp import MultiCoreSim

        sim = MultiCoreSim(nc, NCORES)
        for i in range(NCORES):
            for k, v in in_maps[i].items():
                sim.cores[i].tensor(k)[:] = v
        sim.simulate()
        out = _unpermute(
            meta,
            [np.asarray(sim.cores[i].tensor("out_shard")) for i in range(NCORES)],
        )
        return out

    from concourse.bass_utils import run_bass_kernel_spmd

    trace = bool(int(os.environ.get("KERNEL_TRACE", "0")))
    if trace:
        try:
            import ntff_shim  # noqa: F401
        except ImportError:
            pass

    br = run_bass_kernel_spmd(nc, in_maps, list(range(NCORES)), trace=trace)
    kernel.last_result = br

    out = _unpermute(meta, [r["out_shard"] for r in br.results])
    return out



# revision 48
# speedup vs baseline: 1.1707x; 1.0251x over previous
"""Two-layer GCN + linear head on 8 Trainium2 NeuronCores (Bass/Tile).

Math (per GCN layer, PyG GCNConv with self loops, symmetric norm):
    deg[c]  = 1 + |{e : col_e == c}|          (self loop counted)
    dinv    = 1/sqrt(deg)
    u       = dinv * (x @ W)                  (row-wise pre-scale)
    out[c]  = sum_{e->c} dinv[c] * u[row_e] + dinv[c]^2 * (x @ W)[c] + b
    x1      = relu(out)

Device mapping:
  - Nodes padded to a multiple of 8*128; dst blocks of 128 nodes sharded
    contiguously across 8 cores (49 blocks/core for N=50000).
  - Dense phase sharded: each core computes u for its own blocks in bf16;
    TWO half AllGathers per layer (fired as soon as each half of the dense
    outputs is ready) build the full bf16 table, split in two DRAM halves
    that each fit the int16 dma_gather index range.
  - Scatter phase: edges sorted by (dst block, src half); per 128-edge tile,
    dma_gather (round-robin over 4 SWDGE queues, issued in bursts AHEAD
    chunks early so all queues fill) pulls bf16 u rows; a PURE 0/1 one-hot
    O[e,d] = (col_local_e==d) is built fresh by DVE in both layers (cheap
    single-op is_equal; no DRAM cache). The dst-side dinv factor is
    deferred: folded into the per-partition activation scales of the next
    dense stage (dinv^2 for layer-2's table) and the head's final copy;
    biases enter via rank-1 matmuls with a sqrt(deg) row so the deferred
    scale cancels.
  - Self loops: per block one matmul against a constant 0/1 identity
    (u already carries the src-side dinv factor).
  - Transposed accumulation [f,d] feeds layer-2 dense and the head directly
    as matmul stationary operands (no transposes anywhere).

Host does only index-side prep (shard/sort/pad edge lists, integer degree
counts) — all float math runs on device.
"""
import os
import sys

sys.path.insert(0, "/opt/trn_rl_repo")

import numpy as np

import ml_dtypes

P = 128
NCORES = 8
CH = 8       # tiles per dma_gather call (8*128 = 1024 idxs, SWDGE limit)
AHEAD = 12   # gather chunks issued ahead of consumption (per stream)
GBUFS = AHEAD + 4
NQ = 4       # SWDGE queues, round-robin over gather calls
RUNWAY = 16  # blocks opened ahead (self-loop + stream-0 tiles) per layer
XL = 8       # x^T blocks loaded per DMA in the dense phase


def _ceil_div(a, b):
    return (a + b - 1) // b


def _prep(x, edge_index):
    """Host-side index prep. Returns per-core input dicts + metadata."""
    N, D = x.shape
    assert D == P
    NB = _ceil_div(N, P)
    NB = _ceil_div(NB, NCORES) * NCORES  # blocks multiple of 8
    Npad = NB * P
    NBC = NB // NCORES
    NBH = [min(_ceil_div(NBC, 2), 32768 // (NCORES * P)), 0]
    NBH[1] = NBC - NBH[0]
    assert NBH[1] * NCORES * P <= 32768
    HSZ = [NBH[0] * P, NBH[1] * P]  # per-core rows in each half table
    assert NCORES * HSZ[0] <= 32768 and NCORES * HSZ[1] <= 32768

    row = np.asarray(edge_index[0], dtype=np.int64)
    col = np.asarray(edge_index[1], dtype=np.int64)

    deg = np.bincount(col, minlength=Npad).astype(np.int64) + 1  # + self loop
    deg[N:] = 1

    blk = (col >> 7).astype(np.int64)
    core = blk // NBC
    loc = blk % NBC
    csrc = row // (NBC * P)
    j = row - csrc * (NBC * P)

    # per (core, local block) totals -> per-core block permutation sorted by
    # edge count so the cross-core slot max is tight
    keyb = core * NBC + loc
    tot = np.bincount(keyb, minlength=NCORES * NBC).reshape(NCORES, NBC)
    perm = np.argsort(-tot, axis=1, kind="stable")  # [core, slot] -> local blk
    inv = np.empty_like(perm)
    np.put_along_axis(inv, perm, np.arange(NBC)[None, :], axis=1)

    # source table position: per-core slot order, split in two halves
    spos = inv[csrc, j // P]
    stream = (spos >= NBH[0]).astype(np.int64)
    gidx = np.where(
        stream == 0,
        csrc * HSZ[0] + spos * P + (j % P),
        csrc * HSZ[1] + (spos - NBH[0]) * P + (j % P),
    )
    sloc = inv[core, loc]  # dst slot index

    # order edges by (dst core, dst slot, src half, src)
    order = np.lexsort((gidx, stream, sloc, core))
    gidx_s, col_s = gidx[order], col[order]
    core_s, loc_s, str_s = core[order], sloc[order], stream[order]

    # per (core, dst slot, stream) counts -> shared slot table
    key = (core_s * NBC + loc_s) * 2 + str_s
    cnt = np.bincount(key, minlength=NCORES * NBC * 2).reshape(NCORES, NBC, 2)
    slots = _ceil_div(cnt, P).max(axis=0)  # [NBC(slots), 2]
    ntiles = [int(slots[:, s].sum()) for s in (0, 1)]
    tstart = np.zeros((NBC + 1, 2), dtype=np.int64)
    tstart[1:] = np.cumsum(slots, axis=0)

    starts = np.zeros(NCORES * NBC * 2 + 1, dtype=np.int64)
    starts[1:] = np.cumsum(cnt.reshape(-1))

    cores = []
    for c in range(NCORES):
        idx = [np.zeros(max(ntiles[s], 1) * P, dtype=np.int64) for s in (0, 1)]
        colv = [np.full(max(ntiles[s], 1) * P, 999, dtype=np.int64) for s in (0, 1)]
        for i in range(NBC):
            for s in (0, 1):
                k = (c * NBC + i) * 2 + s
                lo, hi = starts[k], starts[k + 1]
                o0 = tstart[i, s] * P
                idx[s][o0 : o0 + hi - lo] = gidx_s[lo:hi]
                colv[s][o0 : o0 + hi - lo] = col_s[lo:hi] & 127
        colstream = np.concatenate([colv[0][: ntiles[0] * P], colv[1][: ntiles[1] * P]])

        def wrap16(v):  # [ntiles*128] -> [16, n/16] replicated to 128 rows
            w = v.astype(np.int16).reshape(-1, 16).T
            return np.tile(w, (8, 1)).copy()

        def lanes(v, dt):  # [ntiles*128] -> [128, ntiles] (lane-major columns)
            return np.ascontiguousarray(v.reshape(-1, P).T.astype(dt))

        own = deg[c * NBC * P : (c + 1) * NBC * P].reshape(NBC, P)
        deg_own = own[perm[c]].reshape(NBC, P).T.astype(np.float32)  # [128, NBC]

        xp = np.zeros((NBC, P, P), dtype=np.float32)
        realn = min(max(N - c * NBC * P, 0), NBC * P)
        xp.reshape(-1, P)[:realn] = x[c * NBC * P : c * NBC * P + realn]
        xpad = xp[perm[c]].reshape(NBC * P, P)

        cores.append(
            dict(
                xT_shard=np.ascontiguousarray(xpad.T).astype(ml_dtypes.bfloat16),
                idxA=wrap16(idx[0]),
                idxB=wrap16(idx[1]),
                colstream=lanes(colstream, np.float32),
                deg_own=np.ascontiguousarray(deg_own),  # [128, NBC]
                rdeg=np.sqrt(deg_own.T).reshape(1, NBC * P).astype(
                    ml_dtypes.bfloat16
                ),  # [1, NBC*P] sqrt(deg) row, slot order
            )
        )
    meta = dict(
        N=N, Npad=Npad, NB=NB, NBC=NBC, NBH=NBH, HSZ=HSZ,
        ntilesA=ntiles[0], ntilesB=ntiles[1],
        slots=slots, tstart=tstart, perm=perm,
    )
    return cores, meta


def _build_program(meta, with_bias_gcn):
    """Emit the SPMD bass program (identical for all cores)."""
    from concourse import bacc, mybir
    from concourse.tile import TileContext
    from contextlib import ExitStack

    f32 = mybir.dt.float32
    bf16 = mybir.dt.bfloat16
    f8 = mybir.dt.float8e4
    i16 = mybir.dt.int16
    i32 = mybir.dt.int32
    AF = mybir.ActivationFunctionType
    OP = mybir.AluOpType

    NBC, NBH, HSZ = meta["NBC"], meta["NBH"], meta["HSZ"]
    nA, nB = meta["ntilesA"], meta["ntilesB"]
    ntiles = [nA, nB]
    slots = meta["slots"]
    tstart = meta["tstart"]
    ncols = nA + nB
    nchunks = [_ceil_div(nA, CH), _ceil_div(nB, CH)]

    nc = bacc.Bacc(
        "TRN2",
        target_bir_lowering=False,
        num_devices=NCORES,
        # SBUF descriptor carveout: 64KB/partition (ring = 4096 descs/queue,
        # ~4 in-flight 1024-idx gathers per queue) — frees 64KB of SBUF for
        # deeper gather/one-hot pipeline pools vs the 128KB default.
        dynamic_dma_scratch_size=65536,
        num_swdge_queues=NQ,
    )

    xT = nc.declare_dram_parameter("xT_shard", [P, NBC * P], bf16, isOutput=False)
    W1d = nc.declare_dram_parameter("W1", [P, P], bf16, isOutput=False)
    W2d = nc.declare_dram_parameter("W2", [P, P], bf16, isOutput=False)
    Wld = nc.declare_dram_parameter("Wl", [2 * P, P], bf16, isOutput=False)
    b1d = nc.declare_dram_parameter("b1", [1, P], bf16, isOutput=False)
    b2d = nc.declare_dram_parameter("b2", [1, P], bf16, isOutput=False)
    bld = nc.declare_dram_parameter("bl", [1, P], bf16, isOutput=False)
    idxAd = nc.declare_dram_parameter("idxA", [P, max(nA, 1) * 8], i16, isOutput=False)
    idxBd = nc.declare_dram_parameter("idxB", [P, max(nB, 1) * 8], i16, isOutput=False)
    cold = nc.declare_dram_parameter("colstream", [P, ncols], f32, isOutput=False)
    degod = nc.declare_dram_parameter("deg_own", [P, NBC], f32, isOutput=False)
    rdegd = nc.declare_dram_parameter("rdeg", [1, NBC * P], bf16, isOutput=False)
    outd = nc.declare_dram_parameter("out_shard", [NBC * P, P], f32, isOutput=True)

    ag_in = [
        [nc.dram_tensor(f"ag{L}_in_h{h}", [HSZ[h], P], bf16) if NBH[h] else None
         for h in (0, 1)]
        for L in (0, 1)
    ]
    ag_out = [
        [
            nc.dram_tensor(
                f"ag{L}_out_h{h}", [NCORES * HSZ[h], P], bf16, addr_space="Shared"
            ) if NBH[h] else None
            for h in (0, 1)
        ]
        for L in (0, 1)
    ]

    def _emit(tc, ctx):
        const = ctx.enter_context(tc.tile_pool(name="const", bufs=1))
        sb = ctx.enter_context(tc.tile_pool(name="sb", bufs=3))
        gbufs = ctx.enter_context(tc.tile_pool(name="gbufs", bufs=GBUFS))
        ohp = ctx.enter_context(tc.tile_pool(name="ohp", bufs=96))
        psum = ctx.enter_context(tc.tile_pool(name="psum", bufs=6, space="PSUM"))
        psd = ctx.enter_context(tc.tile_pool(name="psd", bufs=1, space="PSUM"))

        # --- constants / streams ---
        iota_i = const.tile([P, P], i32)
        nc.gpsimd.iota(iota_i[:], pattern=[[1, P]], base=0, channel_multiplier=0)
        iota_bf = const.tile([P, P], bf16)
        nc.vector.tensor_copy(out=iota_bf[:], in_=iota_i[:])
        lane_i = const.tile([P, 1], i32)
        nc.gpsimd.iota(lane_i[:], pattern=[[1, 1]], base=0, channel_multiplier=1)
        lane_f = const.tile([P, 1], f32)
        nc.vector.tensor_copy(out=lane_f[:], in_=lane_i[:])

        W1 = const.tile([P, P], bf16)
        W2 = const.tile([P, P], bf16)
        Wl = const.tile([P, 2 * P], bf16)
        nc.sync.dma_start(out=W1[:], in_=W1d[:])
        nc.sync.dma_start(out=W2[:], in_=W2d[:])
        nc.sync.dma_start(out=Wl[:, 0:P], in_=Wld[0:P, :])
        nc.sync.dma_start(out=Wl[:, P : 2 * P], in_=Wld[P : 2 * P, :])

        # bias tiles (row 0 = bias vector)
        btile = []
        for bi, bd in enumerate((b1d, b2d, bld)):
            t = const.tile([P, P], bf16, tag=f"bias{bi}", name=f"bias{bi}")
            nc.vector.memset(t[:], 0.0)
            nc.sync.dma_start(out=t[0:1, :], in_=bd[:])
            btile.append(t)

        # idx/col/rdeg loads are EMITTED after the dense loop (below) so the
        # dense phase's xT loads aren't queued behind ~1.7MB of stream data
        idxs = [
            const.tile([P, max(n, 1) * 8], i16, tag=f"idx{s}", name=f"idx{s}")
            for s, n in ((0, nA), (1, nB))
        ]
        colst = const.tile([P, ncols], f32)

        dinvo = const.tile([P, NBC], f32)
        nc.sync.dma_start(out=dinvo[:], in_=degod[:])
        nc.scalar.activation(out=dinvo[:], in_=dinvo[:], func=AF.Sqrt)
        nc.vector.reciprocal(out=dinvo[:], in_=dinvo[:])
        dinvo2 = const.tile([P, NBC], f32)  # dinv^2, layer-2 dense scale
        nc.scalar.activation(out=dinvo2[:], in_=dinvo[:], func=AF.Square)

        # sqrt(deg) row (partition 0), slot order; [1, 128] slices feed
        # rank-1 bias matmuls (loaded after the dense loop)
        rdeg = const.tile([1, NBC * P], bf16)

        # constant 0/1 identity (self-loop rhs)
        ident = const.tile([P, P], bf16)
        nc.vector.tensor_single_scalar(
            out=ident[:], in_=iota_bf[:], scalar=lane_f[:, 0:1], op=OP.is_equal
        )

        # persistent per-core tiles
        u_bf = const.tile([P, NBC * P], bf16)  # u blocks [node, f]
        x1T = const.tile([P, NBC * P], bf16)   # a1 = relu(acc1) [f, node]

        def dense_block(b, src_lhsT, W, layer):
            """u[b] = scale_own[b] * (src @ W) -> u_bf (bf16, [node, f])."""
            ps = psd.tile([P, P], f32, space="PSUM", tag="psd")
            nc.tensor.matmul(ps[:], lhsT=src_lhsT, rhs=W[:], start=True, stop=True)
            nc.scalar.activation(
                out=u_bf[:, b * P : (b + 1) * P], in_=ps[:], func=AF.Copy,
                scale=(dinvo if layer == 0 else dinvo2)[:, b : b + 1],
            )

        def send_half(layer, h):
            """DMA u_bf half -> ag_in, AllGather into the half table."""
            c0 = 0 if h == 0 else NBH[0]
            nb = NBH[h]
            src = u_bf[:, c0 * P : (c0 + nb) * P].rearrange("p (i f) -> p i f", f=P)
            dst = ag_in[layer][h][:].rearrange("(i p) f -> p i f", p=P)
            nc.sync.dma_start(out=dst, in_=src)
            nc.gpsimd.collective_compute(
                "AllGather", mybir.AluOpType.bypass,
                replica_groups=[list(range(NCORES))],
                ins=[ag_in[layer][h][:]], outs=[ag_out[layer][h][:]],
            )

        qctr = [0]

        def scatter_layer(layer, post_fn):
            """Message passing for one layer; post_fn(b, acc) consumes the
            accumulated transposed block. Gathers are issued AHEAD chunks
            early, round-robin over the SWDGE queues. Both layers build the
            0/1 fp8 one-hots on DVE into a small rotating pool (cheap:
            single-op is_equal against the iota constant)."""
            issued = [[], []]  # stream -> list of gbuf tiles

            def ensure(s, cid):
                # hysteresis: only top up when close to starvation, then issue
                # a burst — batched gather dispatches keep all 4 SWDGE queues
                # busy concurrently instead of trickling one call at a time
                if len(issued[s]) > min(cid + 4, nchunks[s] - 1):
                    return
                while len(issued[s]) <= min(cid + AHEAD, nchunks[s] - 1):
                    c0 = len(issued[s])
                    ch = min(CH, ntiles[s] - c0 * CH)
                    g = gbufs.tile([P, CH, P], bf16, tag=f"g{s}")
                    nc.gpsimd.dma_gather(
                        out_ap=g[:, 0:ch, :],
                        in_ap=ag_out[layer][s][:],
                        idxs_ap=idxs[s][:, c0 * CH * 8 : (c0 * CH + ch) * 8],
                        num_idxs=ch * P,
                        num_idxs_reg=ch * P,
                        elem_size=P,
                        queue_num=qctr[0] % NQ,
                    )
                    qctr[0] += 1
                    issued[s].append(g)

            def oh_src(gcol):
                o = ohp.tile([P, P], bf16, tag="oh")
                nc.vector.tensor_single_scalar(
                    out=o[:], in_=iota_bf[:],
                    scalar=colst[:, gcol : gcol + 1], op=OP.is_equal,
                )
                return o[:]

            def emit_tiles(b, s, acc, k, nmm, stop_last):
                for t in range(tstart[b, s], tstart[b + 1, s]):
                    cid = t // CH
                    ensure(s, cid)
                    g = issued[s][cid]
                    o = oh_src(nA * s + t)
                    k += 1
                    nc.tensor.matmul(
                        acc, lhsT=g[:, t % CH, :], rhs=o,
                        start=False, stop=(stop_last and k == nmm),
                    )
                return k

            state = {}
            acc4s = {}

            def open_block(b):
                """Open the PSUM bank + self-loop matmul only. Tile matmuls
                (and their gather issuance) are deferred to close_block so the
                gpsimd gather queue stays in consumption order — emitting
                far-future stream-A gathers here made the in-order queue head
                wait on far-future matmuls, stalling every gather behind it."""
                if b % 4 == 0:
                    acc4s[b // 4] = psum.tile(
                        [P, 4 * P], f32, space="PSUM", tag="acc4", name="acc4"
                    )
                acc = acc4s[b // 4][:, (b % 4) * P : (b % 4 + 1) * P]
                nmm = int(slots[b, 0] + slots[b, 1])
                # one accumulation group per PSUM bank (4 blocks): start only
                # on the bank's first matmul, stop only on its last
                nc.tensor.matmul(
                    acc, lhsT=u_bf[:, b * P : (b + 1) * P], rhs=ident[:],
                    start=(b % 4 == 0), stop=False,
                )
                state[b] = (acc, 0, nmm)

            def close_block(b):
                acc, k, nmm = state.pop(b)
                bank_last = (b % 4 == 3) or (b == NBC - 1)
                k = emit_tiles(b, 0, acc, k, nmm, False)
                if with_bias_gcn or slots[b, 1] == 0:
                    emit_tiles(b, 1, acc, k, nmm, False)
                    # rank-1: acc[f,d] += bias[f] * sqrt(deg)[d] (cancels the
                    # deferred dinv[d]); zero bias rows make this a no-op
                    # stop-carrier when stream B is empty.
                    nc.tensor.matmul(
                        acc, lhsT=btile[layer][0:1, :],
                        rhs=rdeg[0:1, b * P : (b + 1) * P],
                        start=False, stop=bank_last,
                    )
                else:
                    emit_tiles(b, 1, acc, k, nmm, bank_last)
                if bank_last:
                    k4 = b // 4
                    a4 = acc4s.pop(k4)
                    for bb in range(k4 * 4, b + 1):
                        post_fn(bb, a4[:, (bb % 4) * P : (bb % 4 + 1) * P])

            # prime the gather pipeline: issue the first AHEAD chunks of both
            # streams before any consumer matmuls, so all 4 SWDGE queues fill
            ensure(0, 0)
            if nchunks[1]:
                ensure(1, 0)

            run = min(RUNWAY, NBC)
            for b in range(run):
                open_block(b)
            for b in range(NBC):
                close_block(b)
                if b + run < NBC:
                    open_block(b + run)

        phase = os.environ.get("KERNEL_PHASE", "full")

        # ---------- layer 1 dense + half AllGathers ----------
        lx = None
        for b in range(NBC):
            if b % XL == 0:
                nxt = min(XL, NBC - b)
                lx = sb.tile([P, XL * P], bf16, tag="xT_in", name="lx")
                nc.sync.dma_start(
                    out=lx[:, 0 : nxt * P], in_=xT[:, b * P : (b + nxt) * P]
                )
            dense_block(b, lx[:, (b % XL) * P : (b % XL + 1) * P], W1, 0)
            if b == NBH[0] - 1:
                send_half(0, 0)
        if NBH[1]:
            send_half(0, 1)
        nc.sync.dma_start(out=idxs[0][:], in_=idxAd[:])
        nc.sync.dma_start(out=idxs[1][:], in_=idxBd[:])
        nc.sync.dma_start(out=colst[:], in_=cold[:])
        nc.sync.dma_start(out=rdeg[:], in_=rdegd[:])
        if phase == "dense":
            for b in range(NBC):
                z = sb.tile([P, P], f32, tag="out_t")
                nc.vector.tensor_copy(out=z[:], in_=u_bf[:, b * P : (b + 1) * P])
                nc.sync.dma_start(out=outd[b * P : (b + 1) * P, :], in_=z[:])
            return

        # ---------- layer 1 scatter -> x1T = a1 (+ layer 2 dense) ----------
        def post1(b, acc):
            nc.scalar.activation(
                out=x1T[:, b * P : (b + 1) * P], in_=acc, func=AF.Relu
            )
            dense_block(b, x1T[:, b * P : (b + 1) * P], W2, 1)
            if b == NBH[0] - 1:
                send_half(1, 0)
            elif b == NBC - 1 and NBH[1]:
                send_half(1, 1)

        scatter_layer(0, post1)
        if phase == "l1":
            for b in range(NBC):
                z = sb.tile([P, P], f32, tag="out_t")
                nc.vector.tensor_copy(out=z[:], in_=x1T[:, b * P : (b + 1) * P])
                nc.sync.dma_start(out=outd[b * P : (b + 1) * P, :], in_=z[:])
            return

        # ---------- layer 2 scatter -> head ----------
        def post2(b, acc):
            x2T = sb.tile([P, P], bf16, tag="x2T")
            nc.scalar.activation(out=x2T[:], in_=acc, func=AF.Relu)
            ph = psd.tile([P, P], f32, space="PSUM", tag="ph")
            nc.tensor.matmul(
                ph[:], lhsT=x1T[:, b * P : (b + 1) * P], rhs=Wl[:, 0:P],
                start=True, stop=False,
            )
            nc.tensor.matmul(
                ph[:], lhsT=x2T[:], rhs=Wl[:, P : 2 * P], start=False, stop=False
            )
            # rank-1 bias: ph[d,o] += sqrt(deg)[d] * bl[o]; the final copy's
            # dinv[d] scale turns this into + bl and the x1/x2 terms into
            # their properly normalized values.
            nc.tensor.matmul(
                ph[:], lhsT=rdeg[0:1, b * P : (b + 1) * P],
                rhs=btile[2][0:1, :], start=False, stop=True,
            )
            ot = sb.tile([P, P], f32, tag="out_t")
            nc.scalar.activation(
                out=ot[:], in_=ph[:], func=AF.Copy, scale=dinvo[:, b : b + 1]
            )
            nc.sync.dma_start(out=outd[b * P : (b + 1) * P, :], in_=ot[:])

        scatter_layer(1, post2)

    with TileContext(nc) as tc, ExitStack() as ctx:
        _emit(tc, ctx)

    nc.compile()
    return nc


def _unpermute(meta, shards):
    """Scatter per-core slot-ordered out rows back to natural block order."""
    NBC, Npad, N, perm = meta["NBC"], meta["Npad"], meta["N"], meta["perm"]
    out = np.empty((Npad, P), np.float32)
    ob = out.reshape(-1, P, P)
    for c in range(NCORES):
        ob[c * NBC + perm[c]] = shards[c].reshape(NBC, P, P)
    return np.ascontiguousarray(out[:N])


def kernel(x, edge_index, W1, b1, W2, b2, Wl, bl):
    x = np.asarray(x, dtype=np.float32)
    cores, meta = _prep(x, np.asarray(edge_index))
    with_bias_gcn = bool(np.any(b1) or np.any(b2))

    nc = _build_program(meta, with_bias_gcn)

    bf = ml_dtypes.bfloat16
    shared = dict(
        W1=np.asarray(W1, np.float32).astype(bf),
        W2=np.asarray(W2, np.float32).astype(bf),
        Wl=np.asarray(Wl, np.float32).astype(bf),
        b1=np.asarray(b1, np.float32).astype(bf).reshape(1, P),
        b2=np.asarray(b2, np.float32).astype(bf).reshape(1, P),
        bl=np.asarray(bl, np.float32).astype(bf).reshape(1, P),
    )
    in_maps = [{**c, **shared} for c in cores]
    N = meta["N"]

    if os.environ.get("KERNEL_SIM"):
        from concourse.bass_interp import MultiCoreSim

        sim = MultiCoreSim(nc, NCORES)
        for i in range(NCORES):
            for k, v in in_maps[i].items():
                sim.cores[i].tensor(k)[:] = v
        sim.simulate()
        out = _unpermute(
            meta,
            [np.asarray(sim.cores[i].tensor("out_shard")) for i in range(NCORES)],
        )
        return out

    from concourse.bass_utils import run_bass_kernel_spmd

    trace = bool(int(os.environ.get("KERNEL_TRACE", "0")))
    if trace:
        try:
            import ntff_shim  # noqa: F401
        except ImportError:
            pass

    br = run_bass_kernel_spmd(nc, in_maps, list(range(NCORES)), trace=trace)
    kernel.last_result = br

    out = _unpermute(meta, [r["out_shard"] for r in br.results])
    return out



# revision 50
# speedup vs baseline: 1.1883x; 1.0150x over previous
"""Two-layer GCN + linear head on 8 Trainium2 NeuronCores (Bass/Tile).

Math (per GCN layer, PyG GCNConv with self loops, symmetric norm):
    deg[c]  = 1 + |{e : col_e == c}|          (self loop counted)
    dinv    = 1/sqrt(deg)
    u       = dinv * (x @ W)                  (row-wise pre-scale)
    out[c]  = sum_{e->c} dinv[c] * u[row_e] + dinv[c]^2 * (x @ W)[c] + b
    x1      = relu(out)

Device mapping:
  - Nodes padded to a multiple of 8*128; dst blocks of 128 nodes sharded
    contiguously across 8 cores (49 blocks/core for N=50000).
  - Dense phase sharded: each core computes u for its own blocks in bf16;
    TWO half AllGathers per layer (fired as soon as each half of the dense
    outputs is ready) build the full bf16 table, split in two DRAM halves
    that each fit the int16 dma_gather index range.
  - Scatter phase: edges sorted by (dst block, src half); per 128-edge tile,
    dma_gather (round-robin over 4 SWDGE queues, issued in bursts AHEAD
    chunks early so all queues fill) pulls bf16 u rows; a PURE 0/1 one-hot
    O[e,d] = (col_local_e==d) is built fresh by DVE in both layers (cheap
    single-op is_equal; no DRAM cache). The dst-side dinv factor is
    deferred: folded into the per-partition activation scales of the next
    dense stage (dinv^2 for layer-2's table) and the head's final copy;
    biases enter via rank-1 matmuls with a sqrt(deg) row so the deferred
    scale cancels.
  - Self loops: per block one matmul against a constant 0/1 identity
    (u already carries the src-side dinv factor).
  - Transposed accumulation [f,d] feeds layer-2 dense and the head directly
    as matmul stationary operands (no transposes anywhere).

Host does only index-side prep (shard/sort/pad edge lists, integer degree
counts) — all float math runs on device.
"""
import os
import sys

sys.path.insert(0, "/opt/trn_rl_repo")

import numpy as np

import ml_dtypes

P = 128
NCORES = 8
CH = 8       # tiles per dma_gather call (8*128 = 1024 idxs, SWDGE limit)
AHEAD = 8    # gather chunks issued ahead of consumption (per stream)
GBUFS = AHEAD + 8
NQ = 4       # SWDGE queues, round-robin over gather calls
RUNWAY = 16  # blocks opened ahead (self-loop + stream-0 tiles) per layer
XL = 8       # x^T blocks loaded per DMA in the dense phase


def _ceil_div(a, b):
    return (a + b - 1) // b


def _prep(x, edge_index):
    """Host-side index prep. Returns per-core input dicts + metadata."""
    N, D = x.shape
    assert D == P
    NB = _ceil_div(N, P)
    NB = _ceil_div(NB, NCORES) * NCORES  # blocks multiple of 8
    Npad = NB * P
    NBC = NB // NCORES
    NBH = [min(_ceil_div(NBC, 2), 32768 // (NCORES * P)), 0]
    NBH[1] = NBC - NBH[0]
    assert NBH[1] * NCORES * P <= 32768
    HSZ = [NBH[0] * P, NBH[1] * P]  # per-core rows in each half table
    assert NCORES * HSZ[0] <= 32768 and NCORES * HSZ[1] <= 32768

    row = np.asarray(edge_index[0], dtype=np.int64)
    col = np.asarray(edge_index[1], dtype=np.int64)

    deg = np.bincount(col, minlength=Npad).astype(np.int64) + 1  # + self loop
    deg[N:] = 1

    blk = (col >> 7).astype(np.int64)
    core = blk // NBC
    loc = blk % NBC
    csrc = row // (NBC * P)
    j = row - csrc * (NBC * P)

    # per (core, local block) totals -> per-core block permutation sorted by
    # edge count so the cross-core slot max is tight
    keyb = core * NBC + loc
    tot = np.bincount(keyb, minlength=NCORES * NBC).reshape(NCORES, NBC)
    perm = np.argsort(-tot, axis=1, kind="stable")  # [core, slot] -> local blk
    inv = np.empty_like(perm)
    np.put_along_axis(inv, perm, np.arange(NBC)[None, :], axis=1)

    # source table position: per-core slot order, split in two halves
    spos = inv[csrc, j // P]
    stream = (spos >= NBH[0]).astype(np.int64)
    gidx = np.where(
        stream == 0,
        csrc * HSZ[0] + spos * P + (j % P),
        csrc * HSZ[1] + (spos - NBH[0]) * P + (j % P),
    )
    sloc = inv[core, loc]  # dst slot index

    # order edges by (dst core, dst slot, src half, src)
    order = np.lexsort((gidx, stream, sloc, core))
    gidx_s, col_s = gidx[order], col[order]
    core_s, loc_s, str_s = core[order], sloc[order], stream[order]

    # per (core, dst slot, stream) counts -> shared slot table
    key = (core_s * NBC + loc_s) * 2 + str_s
    cnt = np.bincount(key, minlength=NCORES * NBC * 2).reshape(NCORES, NBC, 2)
    slots = _ceil_div(cnt, P).max(axis=0)  # [NBC(slots), 2]
    ntiles = [int(slots[:, s].sum()) for s in (0, 1)]
    tstart = np.zeros((NBC + 1, 2), dtype=np.int64)
    tstart[1:] = np.cumsum(slots, axis=0)

    starts = np.zeros(NCORES * NBC * 2 + 1, dtype=np.int64)
    starts[1:] = np.cumsum(cnt.reshape(-1))

    cores = []
    for c in range(NCORES):
        idx = [np.zeros(max(ntiles[s], 1) * P, dtype=np.int64) for s in (0, 1)]
        colv = [np.full(max(ntiles[s], 1) * P, 999, dtype=np.int64) for s in (0, 1)]
        for i in range(NBC):
            for s in (0, 1):
                k = (c * NBC + i) * 2 + s
                lo, hi = starts[k], starts[k + 1]
                o0 = tstart[i, s] * P
                idx[s][o0 : o0 + hi - lo] = gidx_s[lo:hi]
                colv[s][o0 : o0 + hi - lo] = col_s[lo:hi] & 127
        colstream = np.concatenate([colv[0][: ntiles[0] * P], colv[1][: ntiles[1] * P]])

        def wrap16(v):  # [ntiles*128] -> [16, n/16] replicated to 128 rows
            w = v.astype(np.int16).reshape(-1, 16).T
            return np.tile(w, (8, 1)).copy()

        def lanes(v, dt):  # [ntiles*128] -> [128, ntiles] (lane-major columns)
            return np.ascontiguousarray(v.reshape(-1, P).T.astype(dt))

        own = deg[c * NBC * P : (c + 1) * NBC * P].reshape(NBC, P)
        deg_own = own[perm[c]].reshape(NBC, P).T.astype(np.float32)  # [128, NBC]

        xp = np.zeros((NBC, P, P), dtype=np.float32)
        realn = min(max(N - c * NBC * P, 0), NBC * P)
        xp.reshape(-1, P)[:realn] = x[c * NBC * P : c * NBC * P + realn]
        xpad = xp[perm[c]].reshape(NBC * P, P)

        cores.append(
            dict(
                xT_shard=np.ascontiguousarray(xpad.T).astype(ml_dtypes.bfloat16),
                idxA=wrap16(idx[0]),
                idxB=wrap16(idx[1]),
                colstream=lanes(colstream, np.float32),
                deg_own=np.ascontiguousarray(deg_own),  # [128, NBC]
                rdeg=np.sqrt(deg_own.T).reshape(1, NBC * P).astype(
                    ml_dtypes.bfloat16
                ),  # [1, NBC*P] sqrt(deg) row, slot order
            )
        )
    meta = dict(
        N=N, Npad=Npad, NB=NB, NBC=NBC, NBH=NBH, HSZ=HSZ,
        ntilesA=ntiles[0], ntilesB=ntiles[1],
        slots=slots, tstart=tstart, perm=perm,
    )
    return cores, meta


def _build_program(meta, with_bias_gcn):
    """Emit the SPMD bass program (identical for all cores)."""
    from concourse import bacc, mybir
    from concourse.tile import TileContext
    from contextlib import ExitStack

    f32 = mybir.dt.float32
    bf16 = mybir.dt.bfloat16
    f8 = mybir.dt.float8e4
    i16 = mybir.dt.int16
    i32 = mybir.dt.int32
    AF = mybir.ActivationFunctionType
    OP = mybir.AluOpType

    NBC, NBH, HSZ = meta["NBC"], meta["NBH"], meta["HSZ"]
    nA, nB = meta["ntilesA"], meta["ntilesB"]
    ntiles = [nA, nB]
    slots = meta["slots"]
    tstart = meta["tstart"]
    ncols = nA + nB
    nchunks = [_ceil_div(nA, CH), _ceil_div(nB, CH)]

    nc = bacc.Bacc(
        "TRN2",
        target_bir_lowering=False,
        num_devices=NCORES,
        # SBUF descriptor carveout: 64KB/partition (ring = 4096 descs/queue,
        # ~4 in-flight 1024-idx gathers per queue) — frees 64KB of SBUF for
        # deeper gather/one-hot pipeline pools vs the 128KB default.
        dynamic_dma_scratch_size=65536,
        num_swdge_queues=NQ,
    )

    xT = nc.declare_dram_parameter("xT_shard", [P, NBC * P], bf16, isOutput=False)
    W1d = nc.declare_dram_parameter("W1", [P, P], bf16, isOutput=False)
    W2d = nc.declare_dram_parameter("W2", [P, P], bf16, isOutput=False)
    Wld = nc.declare_dram_parameter("Wl", [2 * P, P], bf16, isOutput=False)
    b1d = nc.declare_dram_parameter("b1", [1, P], bf16, isOutput=False)
    b2d = nc.declare_dram_parameter("b2", [1, P], bf16, isOutput=False)
    bld = nc.declare_dram_parameter("bl", [1, P], bf16, isOutput=False)
    idxAd = nc.declare_dram_parameter("idxA", [P, max(nA, 1) * 8], i16, isOutput=False)
    idxBd = nc.declare_dram_parameter("idxB", [P, max(nB, 1) * 8], i16, isOutput=False)
    cold = nc.declare_dram_parameter("colstream", [P, ncols], f32, isOutput=False)
    degod = nc.declare_dram_parameter("deg_own", [P, NBC], f32, isOutput=False)
    rdegd = nc.declare_dram_parameter("rdeg", [1, NBC * P], bf16, isOutput=False)
    outd = nc.declare_dram_parameter("out_shard", [NBC * P, P], f32, isOutput=True)

    ag_in = [
        [nc.dram_tensor(f"ag{L}_in_h{h}", [HSZ[h], P], bf16) if NBH[h] else None
         for h in (0, 1)]
        for L in (0, 1)
    ]
    ag_out = [
        [
            nc.dram_tensor(
                f"ag{L}_out_h{h}", [NCORES * HSZ[h], P], bf16, addr_space="Shared"
            ) if NBH[h] else None
            for h in (0, 1)
        ]
        for L in (0, 1)
    ]

    def _emit(tc, ctx):
        const = ctx.enter_context(tc.tile_pool(name="const", bufs=1))
        sb = ctx.enter_context(tc.tile_pool(name="sb", bufs=3))
        gbufs = ctx.enter_context(tc.tile_pool(name="gbufs", bufs=GBUFS))
        ohp = ctx.enter_context(tc.tile_pool(name="ohp", bufs=96))
        psum = ctx.enter_context(tc.tile_pool(name="psum", bufs=6, space="PSUM"))
        psd = ctx.enter_context(tc.tile_pool(name="psd", bufs=1, space="PSUM"))

        # --- constants / streams ---
        iota_i = const.tile([P, P], i32)
        nc.gpsimd.iota(iota_i[:], pattern=[[1, P]], base=0, channel_multiplier=0)
        iota_bf = const.tile([P, P], bf16)
        nc.vector.tensor_copy(out=iota_bf[:], in_=iota_i[:])
        lane_i = const.tile([P, 1], i32)
        nc.gpsimd.iota(lane_i[:], pattern=[[1, 1]], base=0, channel_multiplier=1)
        lane_f = const.tile([P, 1], f32)
        nc.vector.tensor_copy(out=lane_f[:], in_=lane_i[:])

        W1 = const.tile([P, P], bf16)
        W2 = const.tile([P, P], bf16)
        Wl = const.tile([P, 2 * P], bf16)
        nc.sync.dma_start(out=W1[:], in_=W1d[:])
        nc.sync.dma_start(out=W2[:], in_=W2d[:])
        nc.sync.dma_start(out=Wl[:, 0:P], in_=Wld[0:P, :])
        nc.sync.dma_start(out=Wl[:, P : 2 * P], in_=Wld[P : 2 * P, :])

        # bias tiles (row 0 = bias vector)
        btile = []
        for bi, bd in enumerate((b1d, b2d, bld)):
            t = const.tile([P, P], bf16, tag=f"bias{bi}", name=f"bias{bi}")
            nc.vector.memset(t[:], 0.0)
            nc.sync.dma_start(out=t[0:1, :], in_=bd[:])
            btile.append(t)

        # idx/col/rdeg loads are EMITTED after the dense loop (below) so the
        # dense phase's xT loads aren't queued behind ~1.7MB of stream data
        idxs = [
            const.tile([P, max(n, 1) * 8], i16, tag=f"idx{s}", name=f"idx{s}")
            for s, n in ((0, nA), (1, nB))
        ]
        colst = const.tile([P, ncols], f32)

        dinvo = const.tile([P, NBC], f32)
        nc.sync.dma_start(out=dinvo[:], in_=degod[:])
        nc.scalar.activation(out=dinvo[:], in_=dinvo[:], func=AF.Sqrt)
        nc.vector.reciprocal(out=dinvo[:], in_=dinvo[:])
        dinvo2 = const.tile([P, NBC], f32)  # dinv^2, layer-2 dense scale
        nc.scalar.activation(out=dinvo2[:], in_=dinvo[:], func=AF.Square)

        # sqrt(deg) row (partition 0), slot order; [1, 128] slices feed
        # rank-1 bias matmuls (loaded after the dense loop)
        rdeg = const.tile([1, NBC * P], bf16)

        # constant 0/1 identity (self-loop rhs)
        ident = const.tile([P, P], bf16)
        nc.vector.tensor_single_scalar(
            out=ident[:], in_=iota_bf[:], scalar=lane_f[:, 0:1], op=OP.is_equal
        )

        # persistent per-core tiles
        u_bf = const.tile([P, NBC * P], bf16)  # u blocks [node, f]
        x1T = const.tile([P, NBC * P], bf16)   # a1 = relu(acc1) [f, node]

        def dense_block(b, src_lhsT, W, layer):
            """u[b] = scale_own[b] * (src @ W) -> u_bf (bf16, [node, f])."""
            ps = psd.tile([P, P], f32, space="PSUM", tag="psd")
            nc.tensor.matmul(ps[:], lhsT=src_lhsT, rhs=W[:], start=True, stop=True)
            nc.scalar.activation(
                out=u_bf[:, b * P : (b + 1) * P], in_=ps[:], func=AF.Copy,
                scale=(dinvo if layer == 0 else dinvo2)[:, b : b + 1],
            )

        def send_half(layer, h):
            """DMA u_bf half -> ag_in, AllGather into the half table."""
            c0 = 0 if h == 0 else NBH[0]
            nb = NBH[h]
            src = u_bf[:, c0 * P : (c0 + nb) * P].rearrange("p (i f) -> p i f", f=P)
            dst = ag_in[layer][h][:].rearrange("(i p) f -> p i f", p=P)
            nc.sync.dma_start(out=dst, in_=src)
            nc.gpsimd.collective_compute(
                "AllGather", mybir.AluOpType.bypass,
                replica_groups=[list(range(NCORES))],
                ins=[ag_in[layer][h][:]], outs=[ag_out[layer][h][:]],
            )

        qctr = [0]

        def scatter_layer(layer, post_fn):
            """Message passing for one layer; post_fn(b, acc) consumes the
            accumulated transposed block. Gathers are issued AHEAD chunks
            early, round-robin over the SWDGE queues. Both layers build the
            0/1 fp8 one-hots on DVE into a small rotating pool (cheap:
            single-op is_equal against the iota constant)."""
            issued = [[], []]  # stream -> list of gbuf tiles

            def ensure(s, cid):
                # hysteresis: only top up when close to starvation, then issue
                # a burst — batched gather dispatches keep all 4 SWDGE queues
                # busy concurrently instead of trickling one call at a time
                if len(issued[s]) > min(cid + 2, nchunks[s] - 1):
                    return
                while len(issued[s]) <= min(cid + AHEAD, nchunks[s] - 1):
                    c0 = len(issued[s])
                    ch = min(CH, ntiles[s] - c0 * CH)
                    g = gbufs.tile([P, CH, P], bf16, tag=f"g{s}")
                    nc.gpsimd.dma_gather(
                        out_ap=g[:, 0:ch, :],
                        in_ap=ag_out[layer][s][:],
                        idxs_ap=idxs[s][:, c0 * CH * 8 : (c0 * CH + ch) * 8],
                        num_idxs=ch * P,
                        num_idxs_reg=ch * P,
                        elem_size=P,
                        queue_num=qctr[0] % NQ,
                    )
                    qctr[0] += 1
                    issued[s].append(g)

            def oh_src(gcol):
                o = ohp.tile([P, P], bf16, tag="oh")
                nc.vector.tensor_single_scalar(
                    out=o[:], in_=iota_bf[:],
                    scalar=colst[:, gcol : gcol + 1], op=OP.is_equal,
                )
                return o[:]

            def emit_tiles(b, s, acc, k, nmm, stop_last):
                for t in range(tstart[b, s], tstart[b + 1, s]):
                    cid = t // CH
                    ensure(s, cid)
                    g = issued[s][cid]
                    o = oh_src(nA * s + t)
                    k += 1
                    nc.tensor.matmul(
                        acc, lhsT=g[:, t % CH, :], rhs=o,
                        start=False, stop=(stop_last and k == nmm),
                    )
                return k

            state = {}
            acc4s = {}

            def open_block(b):
                """Open the PSUM bank + self-loop matmul only. Tile matmuls
                (and their gather issuance) are deferred to close_block so the
                gpsimd gather queue stays in consumption order — emitting
                far-future stream-A gathers here made the in-order queue head
                wait on far-future matmuls, stalling every gather behind it."""
                if b % 4 == 0:
                    acc4s[b // 4] = psum.tile(
                        [P, 4 * P], f32, space="PSUM", tag="acc4", name="acc4"
                    )
                acc = acc4s[b // 4][:, (b % 4) * P : (b % 4 + 1) * P]
                nmm = int(slots[b, 0] + slots[b, 1])
                # one accumulation group per PSUM bank (4 blocks): start only
                # on the bank's first matmul, stop only on its last
                nc.tensor.matmul(
                    acc, lhsT=u_bf[:, b * P : (b + 1) * P], rhs=ident[:],
                    start=(b % 4 == 0), stop=False,
                )
                state[b] = (acc, 0, nmm)

            def close_block(b):
                acc, k, nmm = state.pop(b)
                bank_last = (b % 4 == 3) or (b == NBC - 1)
                k = emit_tiles(b, 0, acc, k, nmm, False)
                if with_bias_gcn or slots[b, 1] == 0:
                    emit_tiles(b, 1, acc, k, nmm, False)
                    # rank-1: acc[f,d] += bias[f] * sqrt(deg)[d] (cancels the
                    # deferred dinv[d]); zero bias rows make this a no-op
                    # stop-carrier when stream B is empty.
                    nc.tensor.matmul(
                        acc, lhsT=btile[layer][0:1, :],
                        rhs=rdeg[0:1, b * P : (b + 1) * P],
                        start=False, stop=bank_last,
                    )
                else:
                    emit_tiles(b, 1, acc, k, nmm, bank_last)
                if bank_last:
                    k4 = b // 4
                    a4 = acc4s.pop(k4)
                    for bb in range(k4 * 4, b + 1):
                        post_fn(bb, a4[:, (bb % 4) * P : (bb % 4 + 1) * P])

            # prime the gather pipeline: issue the first AHEAD chunks of both
            # streams before any consumer matmuls, so all 4 SWDGE queues fill
            ensure(0, 0)
            if nchunks[1]:
                ensure(1, 0)

            run = min(RUNWAY, NBC)
            for b in range(run):
                open_block(b)
            for b in range(NBC):
                close_block(b)
                if b + run < NBC:
                    open_block(b + run)

        phase = os.environ.get("KERNEL_PHASE", "full")

        # ---------- layer 1 dense + half AllGathers ----------
        lx = None
        for b in range(NBC):
            if b % XL == 0:
                nxt = min(XL, NBC - b)
                lx = sb.tile([P, XL * P], bf16, tag="xT_in", name="lx")
                nc.sync.dma_start(
                    out=lx[:, 0 : nxt * P], in_=xT[:, b * P : (b + nxt) * P]
                )
            dense_block(b, lx[:, (b % XL) * P : (b % XL + 1) * P], W1, 0)
            if b == NBH[0] - 1:
                send_half(0, 0)
        if NBH[1]:
            send_half(0, 1)
        nc.sync.dma_start(out=idxs[0][:], in_=idxAd[:])
        nc.sync.dma_start(out=idxs[1][:], in_=idxBd[:])
        nc.sync.dma_start(out=colst[:], in_=cold[:])
        nc.sync.dma_start(out=rdeg[:], in_=rdegd[:])
        if phase == "dense":
            for b in range(NBC):
                z = sb.tile([P, P], f32, tag="out_t")
                nc.vector.tensor_copy(out=z[:], in_=u_bf[:, b * P : (b + 1) * P])
                nc.sync.dma_start(out=outd[b * P : (b + 1) * P, :], in_=z[:])
            return

        # ---------- layer 1 scatter -> x1T = a1 (+ layer 2 dense) ----------
        def post1(b, acc):
            nc.scalar.activation(
                out=x1T[:, b * P : (b + 1) * P], in_=acc, func=AF.Relu
            )
            dense_block(b, x1T[:, b * P : (b + 1) * P], W2, 1)
            if b == NBH[0] - 1:
                send_half(1, 0)
            elif b == NBC - 1 and NBH[1]:
                send_half(1, 1)

        scatter_layer(0, post1)
        if phase == "l1":
            for b in range(NBC):
                z = sb.tile([P, P], f32, tag="out_t")
                nc.vector.tensor_copy(out=z[:], in_=x1T[:, b * P : (b + 1) * P])
                nc.sync.dma_start(out=outd[b * P : (b + 1) * P, :], in_=z[:])
            return

        # ---------- layer 2 scatter -> head ----------
        def post2(b, acc):
            x2T = sb.tile([P, P], bf16, tag="x2T")
            nc.scalar.activation(out=x2T[:], in_=acc, func=AF.Relu)
            ph = psd.tile([P, P], f32, space="PSUM", tag="ph")
            nc.tensor.matmul(
                ph[:], lhsT=x1T[:, b * P : (b + 1) * P], rhs=Wl[:, 0:P],
                start=True, stop=False,
            )
            nc.tensor.matmul(
                ph[:], lhsT=x2T[:], rhs=Wl[:, P : 2 * P], start=False, stop=False
            )
            # rank-1 bias: ph[d,o] += sqrt(deg)[d] * bl[o]; the final copy's
            # dinv[d] scale turns this into + bl and the x1/x2 terms into
            # their properly normalized values.
            nc.tensor.matmul(
                ph[:], lhsT=rdeg[0:1, b * P : (b + 1) * P],
                rhs=btile[2][0:1, :], start=False, stop=True,
            )
            ot = sb.tile([P, P], f32, tag="out_t")
            nc.scalar.activation(
                out=ot[:], in_=ph[:], func=AF.Copy, scale=dinvo[:, b : b + 1]
            )
            nc.sync.dma_start(out=outd[b * P : (b + 1) * P, :], in_=ot[:])

        scatter_layer(1, post2)

    with TileContext(nc) as tc, ExitStack() as ctx:
        _emit(tc, ctx)

    nc.compile()
    return nc


def _unpermute(meta, shards):
    """Scatter per-core slot-ordered out rows back to natural block order."""
    NBC, Npad, N, perm = meta["NBC"], meta["Npad"], meta["N"], meta["perm"]
    out = np.empty((Npad, P), np.float32)
    ob = out.reshape(-1, P, P)
    for c in range(NCORES):
        ob[c * NBC + perm[c]] = shards[c].reshape(NBC, P, P)
    return np.ascontiguousarray(out[:N])


def kernel(x, edge_index, W1, b1, W2, b2, Wl, bl):
    x = np.asarray(x, dtype=np.float32)
    cores, meta = _prep(x, np.asarray(edge_index))
    with_bias_gcn = bool(np.any(b1) or np.any(b2))

    nc = _build_program(meta, with_bias_gcn)

    bf = ml_dtypes.bfloat16
    shared = dict(
        W1=np.asarray(W1, np.float32).astype(bf),
        W2=np.asarray(W2, np.float32).astype(bf),
        Wl=np.asarray(Wl, np.float32).astype(bf),
        b1=np.asarray(b1, np.float32).astype(bf).reshape(1, P),
        b2=np.asarray(b2, np.float32).astype(bf).reshape(1, P),
        bl=np.asarray(bl, np.float32).astype(bf).reshape(1, P),
    )
    in_maps = [{**c, **shared} for c in cores]
    N = meta["N"]

    if os.environ.get("KERNEL_SIM"):
        from concourse.bass_interp import MultiCoreSim

        sim = MultiCoreSim(nc, NCORES)
        for i in range(NCORES):
            for k, v in in_maps[i].items():
                sim.cores[i].tensor(k)[:] = v
        sim.simulate()
        out = _unpermute(
            meta,
            [np.asarray(sim.cores[i].tensor("out_shard")) for i in range(NCORES)],
        )
        return out

    from concourse.bass_utils import run_bass_kernel_spmd

    trace = bool(int(os.environ.get("KERNEL_TRACE", "0")))
    if trace:
        try:
            import ntff_shim  # noqa: F401
        except ImportError:
            pass

    br = run_bass_kernel_spmd(nc, in_maps, list(range(NCORES)), trace=trace)
    kernel.last_result = br

    out = _unpermute(meta, [r["out_shard"] for r in br.results])
    return out



# revision 59
# speedup vs baseline: 1.2280x; 1.0335x over previous
"""Two-layer GCN + linear head on 8 Trainium2 NeuronCores (Bass/Tile).

Math (per GCN layer, PyG GCNConv with self loops, symmetric norm):
    deg[c]  = 1 + |{e : col_e == c}|          (self loop counted)
    dinv    = 1/sqrt(deg)
    u       = dinv * (x @ W)                  (row-wise pre-scale)
    out[c]  = sum_{e->c} dinv[c] * u[row_e] + dinv[c]^2 * (x @ W)[c] + b
    x1      = relu(out)

Device mapping:
  - Nodes padded to a multiple of 8*128; dst blocks of 128 nodes sharded
    contiguously across 8 cores (49 blocks/core for N=50000).
  - Dense phase sharded: each core computes u for its own blocks in bf16;
    TWO half AllGathers per layer (fired as soon as each half of the dense
    outputs is ready) build the full bf16 table, split in two DRAM halves
    that each fit the int16 dma_gather index range.
  - Scatter phase: edges sorted by (dst block, src half); per 128-edge tile,
    dma_gather (round-robin over 4 SWDGE queues, issued in bursts AHEAD
    chunks early so all queues fill) pulls bf16 u rows; a PURE 0/1 one-hot
    O[e,d] = (col_local_e==d) is built fresh by DVE in both layers (cheap
    single-op is_equal; no DRAM cache). The dst-side dinv factor is
    deferred: folded into the per-partition activation scales of the next
    dense stage (dinv^2 for layer-2's table) and the head's final copy;
    biases enter via rank-1 matmuls with a sqrt(deg) row so the deferred
    scale cancels.
  - Self loops: per block one matmul against a constant 0/1 identity
    (u already carries the src-side dinv factor).
  - Transposed accumulation [f,d] feeds layer-2 dense and the head directly
    as matmul stationary operands (no transposes anywhere).

Host does only index-side prep (shard/sort/pad edge lists, integer degree
counts) — all float math runs on device.
"""
import os
import sys

sys.path.insert(0, "/opt/trn_rl_repo")

import numpy as np

import ml_dtypes

P = 128
NCORES = 8
CH = 8       # tiles per dma_gather call (8*128 = 1024 idxs, SWDGE limit)
AHEAD = 8    # gather chunks issued ahead of consumption (per stream)
GBUFS = AHEAD + 8
NQ = 4       # SWDGE queues, round-robin over gather calls
RUNWAY = 20  # blocks opened ahead (self-loop + stream-A tiles) per layer
XL = 8       # x^T blocks loaded per DMA in the dense phase


def _ceil_div(a, b):
    return (a + b - 1) // b


def _prep(x, edge_index):
    """Host-side index prep. Returns per-core input dicts + metadata."""
    N, D = x.shape
    assert D == P
    NB = _ceil_div(N, P)
    NB = _ceil_div(NB, NCORES) * NCORES  # blocks multiple of 8
    Npad = NB * P
    NBC = NB // NCORES
    NBH = [min(_ceil_div(NBC, 2), 32768 // (NCORES * P)), 0]
    NBH[1] = NBC - NBH[0]
    assert NBH[1] * NCORES * P <= 32768
    HSZ = [NBH[0] * P, NBH[1] * P]  # per-core rows in each half table
    assert NCORES * HSZ[0] <= 32768 and NCORES * HSZ[1] <= 32768

    row = np.asarray(edge_index[0], dtype=np.int64)
    col = np.asarray(edge_index[1], dtype=np.int64)

    deg = np.bincount(col, minlength=Npad).astype(np.int64) + 1  # + self loop
    deg[N:] = 1

    blk = (col >> 7).astype(np.int64)
    core = blk // NBC
    loc = blk % NBC
    csrc = row // (NBC * P)
    j = row - csrc * (NBC * P)

    # per (core, local block) totals -> per-core block permutation sorted by
    # edge count so the cross-core slot max is tight
    keyb = core * NBC + loc
    tot = np.bincount(keyb, minlength=NCORES * NBC).reshape(NCORES, NBC)
    perm = np.argsort(-tot, axis=1, kind="stable")  # [core, slot] -> local blk
    inv = np.empty_like(perm)
    np.put_along_axis(inv, perm, np.arange(NBC)[None, :], axis=1)

    # source table position: per-core slot order, split in two halves
    spos = inv[csrc, j // P]
    stream = (spos >= NBH[0]).astype(np.int64)
    gidx = np.where(
        stream == 0,
        csrc * HSZ[0] + spos * P + (j % P),
        csrc * HSZ[1] + (spos - NBH[0]) * P + (j % P),
    )
    sloc = inv[core, loc]  # dst slot index

    # order edges by (dst core, dst slot, src half, src)
    order = np.lexsort((gidx, stream, sloc, core))
    gidx_s, col_s = gidx[order], col[order]
    core_s, loc_s, str_s = core[order], sloc[order], stream[order]

    # per (core, dst slot, stream) counts -> shared slot table
    key = (core_s * NBC + loc_s) * 2 + str_s
    cnt = np.bincount(key, minlength=NCORES * NBC * 2).reshape(NCORES, NBC, 2)
    slots = _ceil_div(cnt, P).max(axis=0)  # [NBC(slots), 2]
    ntiles = [int(slots[:, s].sum()) for s in (0, 1)]
    tstart = np.zeros((NBC + 1, 2), dtype=np.int64)
    tstart[1:] = np.cumsum(slots, axis=0)

    starts = np.zeros(NCORES * NBC * 2 + 1, dtype=np.int64)
    starts[1:] = np.cumsum(cnt.reshape(-1))

    cores = []
    for c in range(NCORES):
        idx = [np.zeros(max(ntiles[s], 1) * P, dtype=np.int64) for s in (0, 1)]
        colv = [np.full(max(ntiles[s], 1) * P, 999, dtype=np.int64) for s in (0, 1)]
        for i in range(NBC):
            for s in (0, 1):
                k = (c * NBC + i) * 2 + s
                lo, hi = starts[k], starts[k + 1]
                o0 = tstart[i, s] * P
                idx[s][o0 : o0 + hi - lo] = gidx_s[lo:hi]
                colv[s][o0 : o0 + hi - lo] = col_s[lo:hi] & 127
        colstream = np.concatenate([colv[0][: ntiles[0] * P], colv[1][: ntiles[1] * P]])

        def wrap16(v):  # [ntiles*128] -> [16, n/16] replicated to 128 rows
            w = v.astype(np.int16).reshape(-1, 16).T
            return np.tile(w, (8, 1)).copy()

        def lanes(v, dt):  # [ntiles*128] -> [128, ntiles] (lane-major columns)
            return np.ascontiguousarray(v.reshape(-1, P).T.astype(dt))

        own = deg[c * NBC * P : (c + 1) * NBC * P].reshape(NBC, P)
        deg_own = own[perm[c]].reshape(NBC, P).T.astype(np.float32)  # [128, NBC]

        xp = np.zeros((NBC, P, P), dtype=np.float32)
        realn = min(max(N - c * NBC * P, 0), NBC * P)
        xp.reshape(-1, P)[:realn] = x[c * NBC * P : c * NBC * P + realn]
        xpad = xp[perm[c]].reshape(NBC * P, P)

        cores.append(
            dict(
                xT_shard=np.ascontiguousarray(xpad.T).astype(ml_dtypes.bfloat16),
                idxA=wrap16(idx[0]),
                idxB=wrap16(idx[1]),
                colstream=lanes(colstream, np.float32),
                deg_own=np.ascontiguousarray(deg_own),  # [128, NBC]
                rdeg=np.sqrt(deg_own.T).reshape(1, NBC * P).astype(
                    ml_dtypes.bfloat16
                ),  # [1, NBC*P] sqrt(deg) row, slot order
            )
        )
    meta = dict(
        N=N, Npad=Npad, NB=NB, NBC=NBC, NBH=NBH, HSZ=HSZ,
        ntilesA=ntiles[0], ntilesB=ntiles[1],
        slots=slots, tstart=tstart, perm=perm,
    )
    return cores, meta


def _build_program(meta, with_bias_gcn):
    """Emit the SPMD bass program (identical for all cores)."""
    from concourse import bacc, mybir
    from concourse.tile import TileContext
    from contextlib import ExitStack

    f32 = mybir.dt.float32
    bf16 = mybir.dt.bfloat16
    f8 = mybir.dt.float8e4
    i16 = mybir.dt.int16
    i32 = mybir.dt.int32
    AF = mybir.ActivationFunctionType
    OP = mybir.AluOpType

    NBC, NBH, HSZ = meta["NBC"], meta["NBH"], meta["HSZ"]
    nA, nB = meta["ntilesA"], meta["ntilesB"]
    ntiles = [nA, nB]
    slots = meta["slots"]
    tstart = meta["tstart"]
    ncols = nA + nB
    nchunks = [_ceil_div(nA, CH), _ceil_div(nB, CH)]

    nc = bacc.Bacc(
        "TRN2",
        target_bir_lowering=False,
        num_devices=NCORES,
        # SBUF descriptor carveout: 64KB/partition (ring = 4096 descs/queue,
        # ~4 in-flight 1024-idx gathers per queue) — frees 64KB of SBUF for
        # deeper gather/one-hot pipeline pools vs the 128KB default.
        dynamic_dma_scratch_size=65536,
        num_swdge_queues=NQ,
    )

    xT = nc.declare_dram_parameter("xT_shard", [P, NBC * P], bf16, isOutput=False)
    W1d = nc.declare_dram_parameter("W1", [P, P], bf16, isOutput=False)
    W2d = nc.declare_dram_parameter("W2", [P, P], bf16, isOutput=False)
    Wld = nc.declare_dram_parameter("Wl", [2 * P, P], bf16, isOutput=False)
    b1d = nc.declare_dram_parameter("b1", [1, P], bf16, isOutput=False)
    b2d = nc.declare_dram_parameter("b2", [1, P], bf16, isOutput=False)
    bld = nc.declare_dram_parameter("bl", [1, P], bf16, isOutput=False)
    idxAd = nc.declare_dram_parameter("idxA", [P, max(nA, 1) * 8], i16, isOutput=False)
    idxBd = nc.declare_dram_parameter("idxB", [P, max(nB, 1) * 8], i16, isOutput=False)
    cold = nc.declare_dram_parameter("colstream", [P, ncols], f32, isOutput=False)
    degod = nc.declare_dram_parameter("deg_own", [P, NBC], f32, isOutput=False)
    rdegd = nc.declare_dram_parameter("rdeg", [1, NBC * P], bf16, isOutput=False)
    outd = nc.declare_dram_parameter("out_shard", [NBC * P, P], f32, isOutput=True)

    ag_in = [
        [nc.dram_tensor(f"ag{L}_in_h{h}", [HSZ[h], P], bf16) if NBH[h] else None
         for h in (0, 1)]
        for L in (0, 1)
    ]
    ag_out = [
        [
            nc.dram_tensor(
                f"ag{L}_out_h{h}", [NCORES * HSZ[h], P], bf16, addr_space="Shared"
            ) if NBH[h] else None
            for h in (0, 1)
        ]
        for L in (0, 1)
    ]

    def _emit(tc, ctx):
        const = ctx.enter_context(tc.tile_pool(name="const", bufs=1))
        sb = ctx.enter_context(tc.tile_pool(name="sb", bufs=3))
        gbufs = ctx.enter_context(tc.tile_pool(name="gbufs", bufs=GBUFS))
        ohp = ctx.enter_context(tc.tile_pool(name="ohp", bufs=96))
        psum = ctx.enter_context(tc.tile_pool(name="psum", bufs=6, space="PSUM"))
        psd = ctx.enter_context(tc.tile_pool(name="psd", bufs=1, space="PSUM"))

        # --- constants / streams ---
        iota_i = const.tile([P, P], i32)
        nc.gpsimd.iota(iota_i[:], pattern=[[1, P]], base=0, channel_multiplier=0)
        iota_bf = const.tile([P, P], bf16)
        nc.vector.tensor_copy(out=iota_bf[:], in_=iota_i[:])
        lane_i = const.tile([P, 1], i32)
        nc.gpsimd.iota(lane_i[:], pattern=[[1, 1]], base=0, channel_multiplier=1)
        lane_f = const.tile([P, 1], f32)
        nc.vector.tensor_copy(out=lane_f[:], in_=lane_i[:])

        W1 = const.tile([P, P], bf16)
        W2 = const.tile([P, P], bf16)
        Wl = const.tile([P, 2 * P], bf16)
        nc.sync.dma_start(out=W1[:], in_=W1d[:])
        nc.sync.dma_start(out=W2[:], in_=W2d[:])
        nc.sync.dma_start(out=Wl[:, 0:P], in_=Wld[0:P, :])
        nc.sync.dma_start(out=Wl[:, P : 2 * P], in_=Wld[P : 2 * P, :])

        # bias tiles (row 0 = bias vector)
        btile = []
        for bi, bd in enumerate((b1d, b2d, bld)):
            t = const.tile([P, P], bf16, tag=f"bias{bi}", name=f"bias{bi}")
            nc.vector.memset(t[:], 0.0)
            nc.sync.dma_start(out=t[0:1, :], in_=bd[:])
            btile.append(t)

        # idx/col/rdeg loads are EMITTED after the dense loop (below) so the
        # dense phase's xT loads aren't queued behind ~1.7MB of stream data
        idxs = [
            const.tile([P, max(n, 1) * 8], i16, tag=f"idx{s}", name=f"idx{s}")
            for s, n in ((0, nA), (1, nB))
        ]
        colst = const.tile([P, ncols], f32)

        dinvo = const.tile([P, NBC], f32)
        nc.sync.dma_start(out=dinvo[:], in_=degod[:])
        nc.scalar.activation(out=dinvo[:], in_=dinvo[:], func=AF.Sqrt)
        nc.vector.reciprocal(out=dinvo[:], in_=dinvo[:])
        dinvo2 = const.tile([P, NBC], f32)  # dinv^2, layer-2 dense scale
        nc.scalar.activation(out=dinvo2[:], in_=dinvo[:], func=AF.Square)

        # sqrt(deg) row (partition 0), slot order; [1, 128] slices feed
        # rank-1 bias matmuls (loaded after the dense loop)
        rdeg = const.tile([1, NBC * P], bf16)

        # constant 0/1 identity (self-loop rhs)
        ident = const.tile([P, P], bf16)
        nc.vector.tensor_single_scalar(
            out=ident[:], in_=iota_bf[:], scalar=lane_f[:, 0:1], op=OP.is_equal
        )

        # persistent per-core tiles
        u_bf = const.tile([P, NBC * P], bf16)  # u blocks [node, f]
        x1T = const.tile([P, NBC * P], bf16)   # a1 = relu(acc1) [f, node]

        def dense_block(b, src_lhsT, W, layer):
            """u[b] = scale_own[b] * (src @ W) -> u_bf (bf16, [node, f])."""
            ps = psd.tile([P, P], f32, space="PSUM", tag="psd")
            nc.tensor.matmul(ps[:], lhsT=src_lhsT, rhs=W[:], start=True, stop=True)
            nc.scalar.activation(
                out=u_bf[:, b * P : (b + 1) * P], in_=ps[:], func=AF.Copy,
                scale=(dinvo if layer == 0 else dinvo2)[:, b : b + 1],
            )

        def send_half(layer, h):
            """DMA u_bf half -> ag_in, AllGather into the half table."""
            c0 = 0 if h == 0 else NBH[0]
            nb = NBH[h]
            src = u_bf[:, c0 * P : (c0 + nb) * P].rearrange("p (i f) -> p i f", f=P)
            dst = ag_in[layer][h][:].rearrange("(i p) f -> p i f", p=P)
            nc.sync.dma_start(out=dst, in_=src)
            nc.gpsimd.collective_compute(
                "AllGather", mybir.AluOpType.bypass,
                replica_groups=[list(range(NCORES))],
                ins=[ag_in[layer][h][:]], outs=[ag_out[layer][h][:]],
            )

        qctr = [0]
        issued = {L: [[], []] for L in (0, 1)}  # layer -> stream -> gbufs

        def ensure(layer, s, cid):
            # hysteresis: only top up when close to starvation, then issue
            # a burst — batched gather dispatches keep all 4 SWDGE queues
            # busy concurrently instead of trickling one call at a time
            lst = issued[layer][s]
            if len(lst) > min(cid + 2, nchunks[s] - 1):
                return
            while len(lst) <= min(cid + AHEAD, nchunks[s] - 1):
                c0 = len(lst)
                ch = min(CH, ntiles[s] - c0 * CH)
                g = gbufs.tile([P, CH, P], bf16, tag=f"g{s}")
                nc.gpsimd.dma_gather(
                    out_ap=g[:, 0:ch, :],
                    in_ap=ag_out[layer][s][:],
                    idxs_ap=idxs[s][:, c0 * CH * 8 : (c0 * CH + ch) * 8],
                    num_idxs=ch * P,
                    num_idxs_reg=ch * P,
                    elem_size=P,
                    queue_num=qctr[0] % NQ,
                )
                qctr[0] += 1
                lst.append(g)

        def scatter_layer(layer, post_fn):
            """Message passing for one layer; post_fn(b, acc) consumes the
            accumulated transposed block. Gathers are issued AHEAD chunks
            early, round-robin over the SWDGE queues. Both layers build the
            0/1 one-hots on DVE into a small rotating pool (cheap single-op
            is_equal against the iota constant)."""

            def oh_src(gcol):
                o = ohp.tile([P, P], bf16, tag="oh")
                nc.vector.tensor_single_scalar(
                    out=o[:], in_=iota_bf[:],
                    scalar=colst[:, gcol : gcol + 1], op=OP.is_equal,
                )
                return o[:]

            def emit_tiles(b, s, acc, k, nmm, stop_last):
                for t in range(tstart[b, s], tstart[b + 1, s]):
                    cid = t // CH
                    ensure(layer, s, cid)
                    g = issued[layer][s][cid]
                    o = oh_src(nA * s + t)
                    k += 1
                    nc.tensor.matmul(
                        acc, lhsT=g[:, t % CH, :], rhs=o,
                        start=False, stop=(stop_last and k == nmm),
                    )
                return k

            state = {}
            acc4s = {}

            def open_block(b):
                """Open the PSUM bank + self-loop matmul only. Tile matmuls
                (and their gather issuance) are deferred to close_block so the
                gpsimd gather queue stays in consumption order — emitting
                far-future stream-A gathers here made the in-order queue head
                wait on far-future matmuls, stalling every gather behind it."""
                if b % 4 == 0:
                    acc4s[b // 4] = psum.tile(
                        [P, 4 * P], f32, space="PSUM", tag="acc4", name="acc4"
                    )
                acc = acc4s[b // 4][:, (b % 4) * P : (b % 4 + 1) * P]
                nmm = int(slots[b, 0] + slots[b, 1])
                # one accumulation group per PSUM bank (4 blocks): start only
                # on the bank's first matmul, stop only on its last
                nc.tensor.matmul(
                    acc, lhsT=u_bf[:, b * P : (b + 1) * P], rhs=ident[:],
                    start=(b % 4 == 0), stop=False,
                )
                state[b] = [acc, 0, nmm, False]

            def emit_a(b):
                st = state[b]
                st[1] = emit_tiles(b, 0, st[0], st[1], st[2], False)
                st[3] = True

            def close_block(b):
                acc, k, nmm, a_done = state.pop(b)
                bank_last = (b % 4 == 3) or (b == NBC - 1)
                if not a_done:
                    k = emit_tiles(b, 0, acc, k, nmm, False)
                if with_bias_gcn or slots[b, 1] == 0:
                    emit_tiles(b, 1, acc, k, nmm, False)
                    # rank-1: acc[f,d] += bias[f] * sqrt(deg)[d] (cancels the
                    # deferred dinv[d]); zero bias rows make this a no-op
                    # stop-carrier when stream B is empty.
                    nc.tensor.matmul(
                        acc, lhsT=btile[layer][0:1, :],
                        rhs=rdeg[0:1, b * P : (b + 1) * P],
                        start=False, stop=bank_last,
                    )
                else:
                    emit_tiles(b, 1, acc, k, nmm, bank_last)
                if bank_last:
                    k4 = b // 4
                    a4 = acc4s.pop(k4)
                    for bb in range(k4 * 4, b + 1):
                        post_fn(bb, a4[:, (bb % 4) * P : (bb % 4 + 1) * P])

            # prime the gather pipeline: issue the first AHEAD chunks of both
            # streams before any consumer matmuls, so all 4 SWDGE queues fill
            ensure(layer, 0, 0)
            if nchunks[1]:
                ensure(layer, 1, 0)

            run = min(RUNWAY, NBC)
            for b in range(run):
                open_block(b)
            # emit the opened blocks' stream-A tiles before ANY stream-B
            # tile: stream A doesn't depend on the layer's final (half-B)
            # AllGather, so the PE keeps a deep runway of work while that
            # collective's rendezvous completes.
            for b in range(run):
                emit_a(b)
            for b in range(NBC):
                close_block(b)
                if b + run < NBC:
                    open_block(b + run)

        phase = os.environ.get("KERNEL_PHASE", "full")

        # ---------- layer 1 dense + half AllGathers ----------
        lx = None
        for b in range(NBC):
            if b % XL == 0:
                nxt = min(XL, NBC - b)
                lx = sb.tile([P, XL * P], bf16, tag="xT_in", name="lx")
                nc.sync.dma_start(
                    out=lx[:, 0 : nxt * P], in_=xT[:, b * P : (b + nxt) * P]
                )
            dense_block(b, lx[:, (b % XL) * P : (b % XL + 1) * P], W1, 0)
            if b == NBH[0] - 1:
                send_half(0, 0)
        if NBH[1]:
            send_half(0, 1)
        nc.sync.dma_start(out=idxs[0][:], in_=idxAd[:])
        nc.sync.dma_start(out=idxs[1][:], in_=idxBd[:])
        nc.sync.dma_start(out=colst[:], in_=cold[:])
        nc.sync.dma_start(out=rdeg[:], in_=rdegd[:])
        if phase == "dense":
            for b in range(NBC):
                z = sb.tile([P, P], f32, tag="out_t")
                nc.vector.tensor_copy(out=z[:], in_=u_bf[:, b * P : (b + 1) * P])
                nc.sync.dma_start(out=outd[b * P : (b + 1) * P, :], in_=z[:])
            return

        # ---------- layer 1 scatter -> x1T = a1 (+ layer 2 dense) ----------
        def post1(b, acc):
            nc.scalar.activation(
                out=x1T[:, b * P : (b + 1) * P], in_=acc, func=AF.Relu
            )
            dense_block(b, x1T[:, b * P : (b + 1) * P], W2, 1)
            if b == NBH[0] - 1:
                send_half(1, 0)
            elif b == NBC - 1 and NBH[1]:
                # prime layer-2 stream-A gathers BEFORE the half-B collective
                # trigger: the in-order gpsimd queue would otherwise hold
                # them hostage behind the AllGather rendezvous (~80us)
                ensure(1, 0, GBUFS - AHEAD - 1)
                send_half(1, 1)

        scatter_layer(0, post1)
        if phase == "l1":
            for b in range(NBC):
                z = sb.tile([P, P], f32, tag="out_t")
                nc.vector.tensor_copy(out=z[:], in_=x1T[:, b * P : (b + 1) * P])
                nc.sync.dma_start(out=outd[b * P : (b + 1) * P, :], in_=z[:])
            return

        # ---------- layer 2 scatter -> head ----------
        def post2(b, acc):
            x2T = sb.tile([P, P], bf16, tag="x2T")
            nc.scalar.activation(out=x2T[:], in_=acc, func=AF.Relu)
            ph = psd.tile([P, P], f32, space="PSUM", tag="ph")
            nc.tensor.matmul(
                ph[:], lhsT=x1T[:, b * P : (b + 1) * P], rhs=Wl[:, 0:P],
                start=True, stop=False,
            )
            nc.tensor.matmul(
                ph[:], lhsT=x2T[:], rhs=Wl[:, P : 2 * P], start=False, stop=False
            )
            # rank-1 bias: ph[d,o] += sqrt(deg)[d] * bl[o]; the final copy's
            # dinv[d] scale turns this into + bl and the x1/x2 terms into
            # their properly normalized values.
            nc.tensor.matmul(
                ph[:], lhsT=rdeg[0:1, b * P : (b + 1) * P],
                rhs=btile[2][0:1, :], start=False, stop=True,
            )
            ot = sb.tile([P, P], f32, tag="out_t")
            nc.scalar.activation(
                out=ot[:], in_=ph[:], func=AF.Copy, scale=dinvo[:, b : b + 1]
            )
            nc.sync.dma_start(out=outd[b * P : (b + 1) * P, :], in_=ot[:])

        scatter_layer(1, post2)

    with TileContext(nc) as tc, ExitStack() as ctx:
        _emit(tc, ctx)

    nc.compile()
    return nc


def _unpermute(meta, shards):
    """Scatter per-core slot-ordered out rows back to natural block order."""
    NBC, Npad, N, perm = meta["NBC"], meta["Npad"], meta["N"], meta["perm"]
    out = np.empty((Npad, P), np.float32)
    ob = out.reshape(-1, P, P)
    for c in range(NCORES):
        ob[c * NBC + perm[c]] = shards[c].reshape(NBC, P, P)
    return np.ascontiguousarray(out[:N])


def kernel(x, edge_index, W1, b1, W2, b2, Wl, bl):
    x = np.asarray(x, dtype=np.float32)
    cores, meta = _prep(x, np.asarray(edge_index))
    with_bias_gcn = bool(np.any(b1) or np.any(b2))

    nc = _build_program(meta, with_bias_gcn)

    bf = ml_dtypes.bfloat16
    shared = dict(
        W1=np.asarray(W1, np.float32).astype(bf),
        W2=np.asarray(W2, np.float32).astype(bf),
        Wl=np.asarray(Wl, np.float32).astype(bf),
        b1=np.asarray(b1, np.float32).astype(bf).reshape(1, P),
        b2=np.asarray(b2, np.float32).astype(bf).reshape(1, P),
        bl=np.asarray(bl, np.float32).astype(bf).reshape(1, P),
    )
    in_maps = [{**c, **shared} for c in cores]
    N = meta["N"]

    if os.environ.get("KERNEL_SIM"):
        from concourse.bass_interp import MultiCoreSim

        sim = MultiCoreSim(nc, NCORES)
        for i in range(NCORES):
            for k, v in in_maps[i].items():
                sim.cores[i].tensor(k)[:] = v
        sim.simulate()
        out = _unpermute(
            meta,
            [np.asarray(sim.cores[i].tensor("out_shard")) for i in range(NCORES)],
        )
        return out

    from concourse.bass_utils import run_bass_kernel_spmd

    trace = bool(int(os.environ.get("KERNEL_TRACE", "0")))
    if trace:
        try:
            import ntff_shim  # noqa: F401
        except ImportError:
            pass

    br = run_bass_kernel_spmd(nc, in_maps, list(range(NCORES)), trace=trace)
    kernel.last_result = br

    out = _unpermute(meta, [r["out_shard"] for r in br.results])
    return out

